# revision 26
# baseline (speedup 1.0000x reference)
"""Trainium2 Bass kernel for single-head attention model (v4: 3-pass fp8).

Reference computation (B=4, S=2048, E=1024, fp32):
    q = query @ Wq + bq;  k = key @ Wk + bk;  v = value @ Wv + bv
    scores = (q @ k^T) / sqrt(E)
    out = softmax(scores, axis=-1) @ v

Sharding: 8 cores; core c handles batch b = c // 2, query-row half
h = c % 2 (1024 q-rows). No collectives.

Algebraic restructure:  scores_ij = x^q_i A x^k_j + g.x^k_j with
A = Wq Wk^T (computed on device in fp8 DoubleRow), g = Wk bq (host);
bk cancels in softmax.  out = (attn @ Xv) @ Wv + bv.

All five matmul groups run fp8e4m3 DoubleRow (0.5 cyc/row at 256-deep
contraction = 4x bf16 MACs/cycle).  The bf16 phases of the previous
version (Q, Z^T, O) instead use a 3-pass residual expansion:
x ~= xh + xl with xh = fp8(s*x), xl = fp8(s*x - xh), and
(ah+al)(bh+bl) ~= ah*bh + ah*bl + al*bh  (al*bl ~ 4e-4, dropped).
All three passes share one PSUM accumulation group (same binary
scale), so each 3-pass group costs 0.75x the bf16 cycles with better
than bf16 accuracy.  Residual splits of device-produced tensors
(A, exp, Z) are drained as: hi = Act fp8 cast of PSUM, lo = DVE
tensor_sub(PSUM, hi).  Xq/Xv/Wv hi+lo pairs are packed on host.

Softmax sums come from fp8 ones(16.0)-row DoubleRow matmuls over the
eh/el tiles (PE, ~0 cost) instead of a DVE add chain, then a K=1
transpose matmul turns the [1,BQ] sums row into per-partition [128,1]
reciprocal inputs.  recip = 1/(16*sums) so the final Act scale folds
the /16 of psO = 16*O.  The last O chunk folds bv into PSUM via a
rank-1 sums_row x bv matmul; other chunks add a broadcast bv on DVE.

Error sources: one-pass fp8 A (wq8*wk8), k8, qt8 -> model rel-err
1.67e-2 (= previous version), PE busy ~105us vs 133us.
"""

import sys

sys.path.insert(0, "/opt/trn_rl_repo")

from contextlib import ExitStack

import ml_dtypes
import numpy as np

import concourse.mybir as mybir
import concourse.tile as tile
from concourse import bacc, bass_utils

BF16 = mybir.dt.bfloat16
FP16 = mybir.dt.float16
F32 = mybir.dt.float32
FP8 = mybir.dt.float8e4
F32R = mybir.dt.float32r
AF = mybir.ActivationFunctionType
DR = mybir.MatmulPerfMode.DoubleRow

B, S, E = 4, 2048, 1024
N_CORES = 8
SQ = S // 2          # q rows per core
BQ = 512             # s_q block width in attention phase
NBLK = SQ // BQ      # 2 blocks
EK = E // 128        # 8 tiles over e/a/c dims
MK = S // 128        # 16 s_k tiles
NP = EK // 2         # 4 pair-tiles over e contraction
SP8 = MK // 2        # 8 pair-tiles over s_k contraction
INV_SCALE = 1.0 / float(np.sqrt(E))

_cached = {}


def _build():
    nc = bacc.Bacc("TRN2", target_bir_lowering=False, debug=False,
                   num_devices=N_CORES)

    # host pre-packed fp8 DoubleRow inputs, partition-major [128, nt*2*cols]
    # (one row per partition => 1 descriptor/partition/DMA, few big DMAs)
    wq8 = nc.dram_tensor("wq8", [128, NP * 2 * E], FP8,
                         kind="ExternalInput").ap()
    wk8 = nc.dram_tensor("wk8", [128, NP * 2 * E], FP8,
                         kind="ExternalInput").ap()
    xq8h = nc.dram_tensor("xq8h", [128, NP * 2 * SQ], FP8,
                          kind="ExternalInput").ap()
    xq8l = nc.dram_tensor("xq8l", [128, NP * 2 * SQ], FP8,
                          kind="ExternalInput").ap()
    xk8 = nc.dram_tensor("xk8", [128, NP * 2 * S], FP8,
                         kind="ExternalInput").ap()
    xv8h = nc.dram_tensor("xv8h", [128, (S // 256) * 2 * E], FP8,
                          kind="ExternalInput").ap()
    xv8l = nc.dram_tensor("xv8l", [128, (S // 256) * 2 * E], FP8,
                          kind="ExternalInput").ap()
    wv8h = nc.dram_tensor("wv8h", [128, NP * 2 * E], FP8,
                          kind="ExternalInput").ap()
    wv8l = nc.dram_tensor("wv8l", [128, NP * 2 * E], FP8,
                          kind="ExternalInput").ap()
    # g = Wk @ bq arranged g_h[p, t] = g[t*128 + p]; two scale variants
    g16h = nc.dram_tensor("g16h", [128, EK], F32, kind="ExternalInput").ap()
    gh2 = nc.dram_tensor("gh2", [128, EK], F32, kind="ExternalInput").ap()
    bvh = nc.dram_tensor("bvh", [1, E], F32, kind="ExternalInput").ap()
    bv16h = nc.dram_tensor("bv16h", [1, E], FP16, kind="ExternalInput").ap()
    out = nc.dram_tensor("out", [SQ, E], F32, kind="ExternalOutput").ap()

    with tile.TileContext(nc) as tc, ExitStack() as top:
        # ---- long-lived pools ----
        consts = top.enter_context(tc.tile_pool(name="consts", bufs=1))
        qtpool = top.enter_context(tc.tile_pool(name="qtpool", bufs=1))
        xkpool = top.enter_context(tc.tile_pool(name="xkpool", bufs=1))
        xvpool = top.enter_context(tc.tile_pool(name="xvpool", bufs=1))
        wvpool = top.enter_context(tc.tile_pool(name="wvpool", bufs=1))

        # single shared PSUM pool: 8 tags x [128,512]f32 = 8 banks
        psp = top.enter_context(tc.tile_pool(name="psp", bufs=1, space="PSUM"))

        qt8_t = [qtpool.tile([128, 2, SQ], FP8, tag=f"qt8{p}", name=f"qt8{p}")
                 for p in range(NP)]
        xk8_t = xkpool.tile([128, NP, 2, S], FP8, tag="xk8", name="xk8")
        xvh_t = xvpool.tile([128, SP8, 2, E], FP8, tag="xvh", name="xvh")
        xvl_t = xvpool.tile([128, SP8, 2, E], FP8, tag="xvl", name="xvl")
        wvh_t = wvpool.tile([128, NP, 2, E], FP8, tag="wvh", name="wvh")
        wvl_t = wvpool.tile([128, NP, 2, E], FP8, tag="wvl", name="wvl")

        with tc.tile_pool(name="wqwk", bufs=1) as wqwkp, \
             tc.tile_pool(name="apool", bufs=1) as apool, \
             tc.tile_pool(name="xqpool", bufs=1) as xqpool:
            wq8_t = wqwkp.tile([128, NP, 2, E], FP8, tag="wq8", name="wq8")
            wk8_t = wqwkp.tile([128, NP, 2, E], FP8, tag="wk8", name="wk8")
            ah_t = [apool.tile([128, 2, E], FP8, tag=f"ah{t}", name=f"ah{t}")
                    for t in range(NP)]
            al_t = [apool.tile([128, 2, E], FP8, tag=f"al{t}", name=f"al{t}")
                    for t in range(NP)]
            xqh_t = xqpool.tile([128, NP, 2, SQ], FP8, tag="xqh", name="xqh")
            xql_t = xqpool.tile([128, NP, 2, SQ], FP8, tag="xql", name="xql")

            ones_r = consts.tile([128, 256], F32)
            ones_f32r = ones_r[:].bitcast(F32R)
            ones8 = consts.tile([128, 2, 128], FP8)  # value 16.0 (exact fp8)
            ones1 = consts.tile([1, 1], FP16)       # moving for K=1 transpose

            # ---- PE warm-up through the DMA lead-in (reads ones_r
            # uninitialized on purpose; values never consumed) ----
            warm = psp.tile([128, 256], F32, tag="ps0", name="warm")
            for _ in range(20):
                nc.tensor.matmul(warm[:], ones_f32r[:, 0:128],
                                 ones_f32r, start=True, stop=True)
            nc.vector.memset(ones_r[:], 1.0)
            nc.vector.memset(ones8[:], 16.0)
            # psT = (16*sums/1024) * 1024 = 16*sums -> recip = 1/(16*sums)
            nc.vector.memset(ones1[:], 1024.0)

            # ---- DMA issue order = consumption order; big partition-major
            # transfers (1 descriptor/partition), split only for pipelining
            qtr = 2 * E
            for hh in range(NP):
                nc.sync.dma_start(wq8_t[:, hh:hh + 1, :, :],
                                  wq8[:, hh * qtr:(hh + 1) * qtr])
                nc.sync.dma_start(wk8_t[:, hh:hh + 1, :, :],
                                  wk8[:, hh * qtr:(hh + 1) * qtr])
            g16_t = consts.tile([128, EK], F32)
            nc.sync.dma_start(g16_t[:], g16h)
            g2_t = consts.tile([128, EK], F32)
            nc.sync.dma_start(g2_t[:], gh2)
            bv_row = consts.tile([1, E], F32)
            nc.sync.dma_start(bv_row[:], bvh)
            bv16_row = consts.tile([1, E], FP16)
            nc.sync.dma_start(bv16_row[:], bv16h)
            bv_bc = consts.tile([128, E], F32)
            nc.gpsimd.partition_broadcast(bv_bc[:], bv_row[:])
            nc.sync.dma_start(xqh_t[:], xq8h)
            nc.sync.dma_start(xql_t[:], xq8l)
            half_k = NP // 2 * 2 * S
            for hh in range(2):
                nc.sync.dma_start(xk8_t[:, hh * 2:(hh + 1) * 2, :, :],
                                  xk8[:, hh * half_k:(hh + 1) * half_k])
            # value-path loads ride the GPSIMD DMA queue so they stream in
            # parallel with the A/Q/K loads on the sync queue
            half_v = SP8 // 2 * 2 * E
            for hh in range(2):
                nc.gpsimd.dma_start(xvh_t[:, hh * 4:(hh + 1) * 4, :, :],
                                    xv8h[:, hh * half_v:(hh + 1) * half_v])
            for hh in range(2):
                nc.gpsimd.dma_start(xvl_t[:, hh * 4:(hh + 1) * 4, :, :],
                                    xv8l[:, hh * half_v:(hh + 1) * half_v])
            nc.gpsimd.dma_start(wvh_t[:], wv8h)
            nc.gpsimd.dma_start(wvl_t[:], wv8l)

            # ====== phase A: psa = Wq Wk^T (x1024), 1-pass fp8 DR;
            # drains split hi/lo: ah = fp8(psa) [Act], al = psa-ah [DVE] ====
            def a_wave(nb, ts_):
                psa = {t: psp.tile([128, 512], F32, tag=f"ps{t}",
                                   name=f"psA{nb}_{t}") for t in ts_}
                for pr in range(NP):
                    for t in ts_:
                        nc.tensor.matmul(
                            psa[t][:],
                            wq8_t[:, pr, :, t * 128:(t + 1) * 128],
                            wk8_t[:, pr, :, nb * 512:(nb + 1) * 512],
                            start=(pr == 0), stop=(pr == NP - 1),
                            perf_mode=DR)
                for t in ts_:
                    hi = ah_t[t // 2][:, t % 2, nb * 512:(nb + 1) * 512]
                    lo = al_t[t // 2][:, t % 2, nb * 512:(nb + 1) * 512]
                    nc.scalar.copy(hi, psa[t][:])
                    nc.vector.tensor_sub(lo, psa[t][:], hi)

            a_wave(0, range(8))
            a_wave(1, range(0, 4))
            a_wave(1, range(4, 8))

            # ====== phase Q: psq = 16*(A^T Xq) via 3-pass DR;
            # qt8 = (psq + 16*g)/32768 ======
            def q_wave(nb, ms_):
                psq = {m: psp.tile([128, 512], F32, tag=f"ps{m}",
                                   name=f"psQ{nb}_{m}") for m in ms_}
                combos = [(ah_t, xqh_t), (al_t, xqh_t), (ah_t, xql_t)]
                for tp in range(NP):
                    for ci, (a_src, x_src) in enumerate(combos):
                        for m in ms_:
                            nc.tensor.matmul(
                                psq[m][:],
                                a_src[tp][:, :, m * 128:(m + 1) * 128],
                                x_src[:, tp, :, nb * 512:(nb + 1) * 512],
                                start=(tp == 0 and ci == 0),
                                stop=(tp == NP - 1 and ci == 2),
                                perf_mode=DR)
                for m in ms_:
                    dst = qt8_t[m // 2][:, m % 2, nb * 512:(nb + 1) * 512]
                    if m % 2 == 0:
                        nc.vector.tensor_scalar(
                            dst, psq[m][:], g16_t[:, m:m + 1], 1.0 / 32768.0,
                            mybir.AluOpType.add, mybir.AluOpType.mult)
                    else:
                        nc.scalar.activation(
                            dst, psq[m][:], AF.Identity,
                            bias=g2_t[:, m:m + 1], scale=1.0 / 32768.0)

            q_wave(0, range(8))
            q_wave(1, range(0, 4))
            q_wave(1, range(4, 8))

        # ====== phase D: attention, blocked over s_q; blk0/blk1 scores
        # interleave ahead of Z^T so exp hi/lo drains stay off PE's path ==
        with tc.tile_pool(name="exsp", bufs=2) as exsp, \
             tc.tile_pool(name="ehlp", bufs=2) as ehlp, \
             tc.tile_pool(name="zhlp", bufs=2) as zhlp, \
             tc.tile_pool(name="otp", bufs=1) as otp, \
             tc.tile_pool(name="srp", bufs=2) as srp, \
             tc.tile_pool(name="rcp", bufs=2) as rcp:
            eh_t = {(blk, tp): ehlp.tile([128, 2, BQ], FP8,
                                         tag=f"eh{tp}", name=f"eh{blk}_{tp}")
                    for blk in range(NBLK) for tp in range(SP8)}
            el_t = {(blk, tp): ehlp.tile([128, 2, BQ], FP8,
                                         tag=f"el{tp}", name=f"el{blk}_{tp}")
                    for blk in range(NBLK) for tp in range(SP8)}
            zh_t = {(blk, zp): zhlp.tile([128, 2, BQ], FP8,
                                         tag=f"zh{zp}", name=f"zh{blk}_{zp}")
                    for blk in range(NBLK) for zp in range(NP)}
            zl_t = {(blk, zp): zhlp.tile([128, 2, BQ], FP8,
                                         tag=f"zl{zp}", name=f"zl{blk}_{zp}")
                    for blk in range(NBLK) for zp in range(NP)}

            def exp_drain(blk, m, pss):
                ex = exsp.tile([128, BQ], F32, tag=f"ex{m % 8}",
                               name=f"ex{blk}_{m}")
                nc.scalar.activation(ex[:], pss[:], AF.Exp,
                                     scale=INV_SCALE / 4.0)
                hi = eh_t[(blk, m // 2)][:, m % 2, :]
                lo = el_t[(blk, m // 2)][:, m % 2, :]
                nc.vector.tensor_copy(hi, ex[:])
                # alternate the residual subtract across DVE/Pool so neither
                # serial stream paces the exp-consuming matmuls
                if m % 2 == 0:
                    nc.vector.tensor_sub(lo, ex[:], hi)
                else:
                    nc.gpsimd.tensor_sub(lo, ex[:], hi)

            # scores blk0: k-outer 8-bank waves (start as xk8[0] lands)
            q0 = 0
            for half in range(2):
                ms = range(half * 8, half * 8 + 8)
                pss = {m: psp.tile([128, BQ], F32, tag=f"ps{m % 8}",
                                   name=f"psS0_{m}") for m in ms}
                for pr in range(NP):
                    for m in ms:
                        nc.tensor.matmul(
                            pss[m][:],
                            xk8_t[:, pr, :, m * 128:(m + 1) * 128],
                            qt8_t[pr][:, :, q0:q0 + BQ],
                            start=(pr == 0), stop=(pr == NP - 1),
                            perf_mode=DR)
                for m in ms:
                    exp_drain(0, m, pss[m])

            # scores blk1: 2-bank ping-pong
            q0 = BQ
            for m in range(MK):
                ps = psp.tile([128, BQ], F32, tag=f"ps{m % 2}",
                              name=f"psS1_{m}")
                for pr in range(NP):
                    nc.tensor.matmul(
                        ps[:],
                        xk8_t[:, pr, :, m * 128:(m + 1) * 128],
                        qt8_t[pr][:, :, q0:q0 + BQ],
                        start=(pr == 0), stop=(pr == NP - 1),
                        perf_mode=DR)
                exp_drain(1, m, ps)

            def zt_phase(blk):
                # Z^T/2 [e, q] = sum_s (Xv/2)^T exp: 3-pass DR, e_-outer.
                # eh-only passes run first so the el drains (one extra hop
                # behind eh) have the whole first loop to land.
                for e_ in range(EK):
                    ps = psp.tile([128, BQ], F32, tag=f"ps{2 + e_ % 2}",
                                  name=f"psZ{blk}_{e_}")
                    sl = slice(e_ * 128, (e_ + 1) * 128)
                    for tp in range(SP8):
                        nc.tensor.matmul(
                            ps[:], xvh_t[:, tp, :, sl], eh_t[(blk, tp)][:],
                            start=(tp == 0), stop=False, perf_mode=DR)
                        nc.tensor.matmul(
                            ps[:], xvl_t[:, tp, :, sl], eh_t[(blk, tp)][:],
                            start=False, stop=False, perf_mode=DR)
                    for tp in range(SP8):
                        nc.tensor.matmul(
                            ps[:], xvh_t[:, tp, :, sl], el_t[(blk, tp)][:],
                            start=False, stop=(tp == SP8 - 1), perf_mode=DR)
                    hi = zh_t[(blk, e_ // 2)][:, e_ % 2, :]
                    lo = zl_t[(blk, e_ // 2)][:, e_ % 2, :]
                    nc.scalar.copy(hi, ps[:])
                    nc.vector.tensor_sub(lo, ps[:], hi)

            def sums_phase(blk):
                # sums row = 16 * colsum(exp) via ones(16) DR matmuls,
                # then K=1 transpose to per-partition recip inputs
                # el's contribution to sums is +-2%/sqrt(2048) ~ 4e-4: skip it
                ps_row = psp.tile([128, BQ], F32, tag="ps6",
                                  name=f"psRow{blk}")
                for tp in range(SP8):
                    nc.tensor.matmul(ps_row[:], ones8[:],
                                     eh_t[(blk, tp)][:],
                                     start=(tp == 0), stop=(tp == SP8 - 1),
                                     perf_mode=DR)
                # fp16 row of 16*sums/1024 (~40, exact to 5e-4); stationary
                # for both the recip transpose and the rank-1 bv fold
                sums_sb = srp.tile([1, BQ], FP16, tag="sums_sb",
                                   name=f"sums_sb{blk}")
                nc.scalar.activation(sums_sb[:], ps_row[0:1, :],
                                     AF.Copy, scale=1.0 / 1024.0)
                sums16 = sums_sb[0:1, BQ - 128:BQ] if blk == NBLK - 1 else None
                psT = psp.tile([128, 4], F32, tag="ps7", name=f"psT{blk}")
                recips = []
                for it in range(BQ // 128):
                    nc.tensor.matmul(psT[:, it:it + 1],
                                     sums_sb[0:1, it * 128:(it + 1) * 128],
                                     ones1[:], start=True, stop=True)
                for it in range(BQ // 128):
                    rc = rcp.tile([128, 1], F32, tag=f"rc{it}",
                                  name=f"rc{blk}_{it}")
                    nc.vector.reciprocal(rc[:], psT[:, it:it + 1])
                    recips.append(rc)
                return recips, sums16

            def o_phase(blk, recips, sums16):
                # psO = 16*O via 3-pass DR over zh/zl x wvh/wvl.  One merged
                # out-DMA per i-tile except the last two (those stream
                # per-chunk so the end-of-kernel DMA chain is short).
                q0 = blk * BQ
                for it in range(BQ // 128):
                    ot = otp.tile([128, E], F32, tag=f"ot{it}",
                                  name=f"ot{blk}_{it}")
                    tail = (blk == NBLK - 1 and it >= BQ // 128 - 2)
                    last_it = (blk == NBLK - 1 and it == BQ // 128 - 1)
                    # taper the very last i-tile so the end-of-kernel
                    # Act->descgen->DMA chain covers only 128 columns
                    widths = [512, 256, 128, 128] if last_it else [512, 512]
                    f0 = 0
                    for ci, cw in enumerate(widths):
                        final = last_it and ci == len(widths) - 1
                        ps = psp.tile([128, cw], F32, tag=f"ps{4 + ci % 2}",
                                      name=f"psO{blk}_{it}_{ci}")
                        isl = slice(it * 128, (it + 1) * 128)
                        combos = [(zh_t, wvh_t), (zh_t, wvl_t), (zl_t, wvh_t)]
                        for zp in range(NP):
                            for k, (z_src, w_src) in enumerate(combos):
                                nc.tensor.matmul(
                                    ps[:],
                                    z_src[(blk, zp)][:, :, isl],
                                    w_src[:, zp, :, f0:f0 + cw],
                                    start=(zp == 0 and k == 0),
                                    stop=(zp == NP - 1 and k == 2
                                          and not final),
                                    perf_mode=DR)
                        if final:
                            # psO += (16*sums/1024) * (1024*bv); after the
                            # recip scale this is exactly + bv
                            nc.tensor.matmul(
                                ps[:], sums16,
                                bv16_row[0:1, f0:f0 + cw],
                                start=False, stop=True)
                        nc.scalar.activation(
                            ot[:, f0:f0 + cw], ps[:],
                            AF.Copy, scale=recips[it][:])
                        if not final:
                            nc.vector.tensor_add(
                                ot[:, f0:f0 + cw],
                                ot[:, f0:f0 + cw],
                                bv_bc[:, f0:f0 + cw])
                        if tail:
                            nc.sync.dma_start(
                                out[q0 + it * 128:q0 + (it + 1) * 128,
                                    f0:f0 + cw],
                                ot[:, f0:f0 + cw])
                        f0 += cw
                    if not tail:
                        nc.sync.dma_start(
                            out[q0 + it * 128:q0 + (it + 1) * 128, :],
                            ot[:])

            zt_phase(0)
            rec0 = sums_phase(0)
            zt_phase(1)
            rec1 = sums_phase(1)
            o_phase(0, *rec0)
            o_phase(1, *rec1)

    nc.compile()
    return nc


def _get_nc():
    if "nc" not in _cached:
        _cached["nc"] = _build()
    return _cached["nc"]


def _split8(x, s):
    """fp8 hi/lo pair of s*x (hi + lo == s*x up to lo's own rounding)."""
    xs = np.asarray(x, np.float32) * s
    hi = xs.astype(ml_dtypes.float8_e4m3)
    lo = (xs - hi.astype(np.float32)).astype(ml_dtypes.float8_e4m3)
    return hi, lo


def _pack_pairs(a):
    """[R, C] fp8 -> DoubleRow pair tiles, partition-major [128, nt*2*C]."""
    r, c = a.shape
    return np.ascontiguousarray(
        a.reshape(r // 256, 2, 128, c).transpose(2, 0, 1, 3)
        .reshape(128, (r // 128) * c))


def kernel(query, key, value, Wq, bq, Wk, bk, Wv, bv, **kw):
    query = np.asarray(query, dtype=np.float32)
    key = np.asarray(key, dtype=np.float32)
    value = np.asarray(value, dtype=np.float32)
    Wq = np.asarray(Wq, dtype=np.float32)
    Wk = np.asarray(Wk, dtype=np.float32)
    Wv = np.asarray(Wv, dtype=np.float32)
    bq = np.asarray(bq, dtype=np.float32)
    bv = np.asarray(bv, dtype=np.float32)

    def _f8_one(x, s):
        return _pack_pairs((np.asarray(x, np.float32) * s)
                           .astype(ml_dtypes.float8_e4m3))

    wq8_h = _f8_one(Wq.T, 32.0)
    wk8_h = _f8_one(Wk.T, 32.0)
    wvh, wvl = _split8(Wv, 32.0)
    wvh_h, wvl_h = _pack_pairs(wvh), _pack_pairs(wvl)

    g_dev = (Wk @ bq) * 1024.0        # [E]; bk cancels in softmax
    g16_h = np.ascontiguousarray(
        (16.0 * g_dev).reshape(EK, 128).T).astype(np.float32)
    gh2_h = np.ascontiguousarray(
        (g_dev / 2048.0).reshape(EK, 128).T).astype(np.float32)
    bv_h = np.ascontiguousarray(bv.reshape(1, E))
    bv16_h = np.ascontiguousarray(
        (1024.0 * bv).reshape(1, E)).astype(np.float16)

    key8 = {}
    val8 = {}
    for b in range(B):
        key8[b] = _f8_one(key[b].T, 8.0)
        vh, vl = _split8(value[b], 0.5)
        val8[b] = (_pack_pairs(vh), _pack_pairs(vl))

    in_maps = []
    for c in range(N_CORES):
        b, h = divmod(c, 2)
        xqh, xql = _split8(query[b, h * SQ:(h + 1) * SQ, :].T, 16.0)
        in_maps.append({
            "wq8": wq8_h, "wk8": wk8_h,
            "xq8h": _pack_pairs(xqh), "xq8l": _pack_pairs(xql),
            "xk8": key8[b], "xv8h": val8[b][0], "xv8l": val8[b][1],
            "wv8h": wvh_h, "wv8l": wvl_h,
            "g16h": g16_h, "gh2": gh2_h, "bvh": bv_h, "bv16h": bv16_h,
        })

    nc = _get_nc()
    res = bass_utils.run_bass_kernel_spmd(
        nc, in_maps, core_ids=list(range(N_CORES)), **kw)

    full = np.empty((B, S, E), dtype=np.float32)
    for c in range(N_CORES):
        b, h = divmod(c, 2)
        full[b, h * SQ:(h + 1) * SQ, :] = res.results[c]["out"]
    kernel.last_results = res
    return full


# revision 28
# speedup vs baseline: 1.0932x; 1.0932x over previous
"""Trainium2 Bass kernel for single-head attention model (v4: 3-pass fp8).

Reference computation (B=4, S=2048, E=1024, fp32):
    q = query @ Wq + bq;  k = key @ Wk + bk;  v = value @ Wv + bv
    scores = (q @ k^T) / sqrt(E)
    out = softmax(scores, axis=-1) @ v

Sharding: 8 cores; core c handles batch b = c // 2, query-row half
h = c % 2 (1024 q-rows). No collectives.

Algebraic restructure:  scores_ij = x^q_i A x^k_j + g.x^k_j with
A = Wq Wk^T (computed on device in fp8 DoubleRow), g = Wk bq (host);
bk cancels in softmax.  out = (attn @ Xv) @ Wv + bv.

All five matmul groups run fp8e4m3 DoubleRow (0.5 cyc/row at 256-deep
contraction = 4x bf16 MACs/cycle).  The bf16 phases of the previous
version (Q, Z^T, O) instead use a 3-pass residual expansion:
x ~= xh + xl with xh = fp8(s*x), xl = fp8(s*x - xh), and
(ah+al)(bh+bl) ~= ah*bh + ah*bl + al*bh  (al*bl ~ 4e-4, dropped).
All three passes share one PSUM accumulation group (same binary
scale), so each 3-pass group costs 0.75x the bf16 cycles with better
than bf16 accuracy.  Residual splits of device-produced tensors
(A, exp, Z) are drained as: hi = Act fp8 cast of PSUM, lo = DVE
tensor_sub(PSUM, hi).  Xq/Xv/Wv hi+lo pairs are packed on host.

Softmax sums come from fp8 ones(16.0)-row DoubleRow matmuls over the
eh/el tiles (PE, ~0 cost) instead of a DVE add chain, then a K=1
transpose matmul turns the [1,BQ] sums row into per-partition [128,1]
reciprocal inputs.  recip = 1/(16*sums) so the final Act scale folds
the /16 of psO = 16*O.  The last O chunk folds bv into PSUM via a
rank-1 sums_row x bv matmul; other chunks add a broadcast bv on DVE.

Error sources: one-pass fp8 A (wq8*wk8), k8, qt8 -> model rel-err
1.67e-2 (= previous version), PE busy ~105us vs 133us.
"""

import sys

sys.path.insert(0, "/opt/trn_rl_repo")

from contextlib import ExitStack

import ml_dtypes
import numpy as np

import concourse.mybir as mybir
import concourse.tile as tile
from concourse import bacc, bass_utils

BF16 = mybir.dt.bfloat16
FP16 = mybir.dt.float16
F32 = mybir.dt.float32
FP8 = mybir.dt.float8e4
F32R = mybir.dt.float32r
AF = mybir.ActivationFunctionType
DR = mybir.MatmulPerfMode.DoubleRow

B, S, E = 4, 2048, 1024
N_CORES = 8
SQ = S // 2          # q rows per core
BQ = 512             # s_q block width in attention phase
NBLK = SQ // BQ      # 2 blocks
EK = E // 128        # 8 tiles over e/a/c dims
MK = S // 128        # 16 s_k tiles
NP = EK // 2         # 4 pair-tiles over e contraction
SP8 = MK // 2        # 8 pair-tiles over s_k contraction
INV_SCALE = 1.0 / float(np.sqrt(E))

_cached = {}


def _build():
    nc = bacc.Bacc("TRN2", target_bir_lowering=False, debug=False,
                   num_devices=N_CORES)

    # host pre-packed fp8 DoubleRow inputs, partition-major [128, nt*2*cols]
    # (one row per partition => 1 descriptor/partition/DMA, few big DMAs)
    wq8 = nc.dram_tensor("wq8", [128, NP * 2 * E], FP8,
                         kind="ExternalInput").ap()
    wk8 = nc.dram_tensor("wk8", [128, NP * 2 * E], FP8,
                         kind="ExternalInput").ap()
    xq8h = nc.dram_tensor("xq8h", [128, NP * 2 * SQ], FP8,
                          kind="ExternalInput").ap()
    xq8l = nc.dram_tensor("xq8l", [128, NP * 2 * SQ], FP8,
                          kind="ExternalInput").ap()
    xk8 = nc.dram_tensor("xk8", [128, NP * 2 * S], FP8,
                         kind="ExternalInput").ap()
    xv8h = nc.dram_tensor("xv8h", [128, (S // 256) * 2 * E], FP8,
                          kind="ExternalInput").ap()
    xv8l = nc.dram_tensor("xv8l", [128, (S // 256) * 2 * E], FP8,
                          kind="ExternalInput").ap()
    wv8h = nc.dram_tensor("wv8h", [128, NP * 2 * E], FP8,
                          kind="ExternalInput").ap()
    wv8l = nc.dram_tensor("wv8l", [128, NP * 2 * E], FP8,
                          kind="ExternalInput").ap()
    # g = Wk @ bq arranged g_h[p, t] = g[t*128 + p]; two scale variants
    g16h = nc.dram_tensor("g16h", [128, EK], F32, kind="ExternalInput").ap()
    gh2 = nc.dram_tensor("gh2", [128, EK], F32, kind="ExternalInput").ap()
    bvh = nc.dram_tensor("bvh", [1, E], F32, kind="ExternalInput").ap()
    bv16h = nc.dram_tensor("bv16h", [1, E], FP16, kind="ExternalInput").ap()
    out = nc.dram_tensor("out", [SQ, E], F32, kind="ExternalOutput").ap()

    with tile.TileContext(nc) as tc, ExitStack() as top:
        # ---- long-lived pools ----
        consts = top.enter_context(tc.tile_pool(name="consts", bufs=1))
        qtpool = top.enter_context(tc.tile_pool(name="qtpool", bufs=1))
        xkpool = top.enter_context(tc.tile_pool(name="xkpool", bufs=1))
        xvpool = top.enter_context(tc.tile_pool(name="xvpool", bufs=1))
        wvpool = top.enter_context(tc.tile_pool(name="wvpool", bufs=1))

        # single shared PSUM pool: 8 tags x [128,512]f32 = 8 banks
        psp = top.enter_context(tc.tile_pool(name="psp", bufs=1, space="PSUM"))

        qt8_t = [qtpool.tile([128, 2, SQ], FP8, tag=f"qt8{p}", name=f"qt8{p}")
                 for p in range(NP)]
        xk8_t = xkpool.tile([128, NP, 2, S], FP8, tag="xk8", name="xk8")
        xvh_t = xvpool.tile([128, SP8, 2, E], FP8, tag="xvh", name="xvh")
        xvl_t = xvpool.tile([128, SP8, 2, E], FP8, tag="xvl", name="xvl")
        wvh_t = wvpool.tile([128, NP, 2, E], FP8, tag="wvh", name="wvh")
        wvl_t = wvpool.tile([128, NP, 2, E], FP8, tag="wvl", name="wvl")

        with tc.tile_pool(name="wqwk", bufs=1) as wqwkp, \
             tc.tile_pool(name="apool", bufs=1) as apool, \
             tc.tile_pool(name="xqpool", bufs=1) as xqpool:
            wq8_t = wqwkp.tile([128, NP, 2, E], FP8, tag="wq8", name="wq8")
            wk8_t = wqwkp.tile([128, NP, 2, E], FP8, tag="wk8", name="wk8")
            ah_t = [apool.tile([128, 2, E], FP8, tag=f"ah{t}", name=f"ah{t}")
                    for t in range(NP)]
            al_t = [apool.tile([128, 2, E], FP8, tag=f"al{t}", name=f"al{t}")
                    for t in range(NP)]
            xqh_t = xqpool.tile([128, NP, 2, SQ], FP8, tag="xqh", name="xqh")
            xql_t = xqpool.tile([128, NP, 2, SQ], FP8, tag="xql", name="xql")

            ones_r = consts.tile([128, 256], F32)
            ones_f32r = ones_r[:].bitcast(F32R)
            ones8 = consts.tile([128, 2, 128], FP8)  # value 16.0 (exact fp8)
            ones1 = consts.tile([1, 1], FP16)       # moving for K=1 transpose

            # ---- PE warm-up through the DMA lead-in (reads ones_r
            # uninitialized on purpose; values never consumed) ----
            warm = psp.tile([128, 256], F32, tag="ps0", name="warm")
            for _ in range(20):
                nc.tensor.matmul(warm[:], ones_f32r[:, 0:128],
                                 ones_f32r, start=True, stop=True)
            nc.vector.memset(ones_r[:], 1.0)
            nc.vector.memset(ones8[:], 16.0)
            # psT = (16*sums/1024) * 1024 = 16*sums -> recip = 1/(16*sums)
            nc.vector.memset(ones1[:], 1024.0)

            # ---- DMA issue order = consumption order; big partition-major
            # transfers (1 descriptor/partition), split only for pipelining
            half_q = NP // 2 * 2 * E
            for hh in range(2):
                nc.sync.dma_start(wq8_t[:, hh * 2:(hh + 1) * 2, :, :],
                                  wq8[:, hh * half_q:(hh + 1) * half_q])
                nc.sync.dma_start(wk8_t[:, hh * 2:(hh + 1) * 2, :, :],
                                  wk8[:, hh * half_q:(hh + 1) * half_q])
            g16_t = consts.tile([128, EK], F32)
            nc.sync.dma_start(g16_t[:], g16h)
            g2_t = consts.tile([128, EK], F32)
            nc.sync.dma_start(g2_t[:], gh2)
            bv_row = consts.tile([1, E], F32)
            nc.sync.dma_start(bv_row[:], bvh)
            bv16_row = consts.tile([1, E], FP16)
            nc.sync.dma_start(bv16_row[:], bv16h)
            bv_bc = consts.tile([128, E], F32)
            nc.gpsimd.partition_broadcast(bv_bc[:], bv_row[:])
            nc.sync.dma_start(xqh_t[:], xq8h)
            nc.sync.dma_start(xql_t[:], xq8l)
            half_k = NP // 2 * 2 * S
            for hh in range(2):
                nc.sync.dma_start(xk8_t[:, hh * 2:(hh + 1) * 2, :, :],
                                  xk8[:, hh * half_k:(hh + 1) * half_k])
            half_v = SP8 // 2 * 2 * E
            for hh in range(2):
                nc.sync.dma_start(xvh_t[:, hh * 4:(hh + 1) * 4, :, :],
                                  xv8h[:, hh * half_v:(hh + 1) * half_v])
            for hh in range(2):
                nc.sync.dma_start(xvl_t[:, hh * 4:(hh + 1) * 4, :, :],
                                  xv8l[:, hh * half_v:(hh + 1) * half_v])
            nc.sync.dma_start(wvh_t[:], wv8h)
            nc.sync.dma_start(wvl_t[:], wv8l)

            # ====== phase A: psa = Wq Wk^T (x1024), 1-pass fp8 DR;
            # drains split hi/lo: ah = fp8(psa) [Act], al = psa-ah [DVE] ====
            def a_wave(nb, ts_):
                psa = {t: psp.tile([128, 512], F32, tag=f"ps{t}",
                                   name=f"psA{nb}_{t}") for t in ts_}
                for pr in range(NP):
                    for t in ts_:
                        nc.tensor.matmul(
                            psa[t][:],
                            wq8_t[:, pr, :, t * 128:(t + 1) * 128],
                            wk8_t[:, pr, :, nb * 512:(nb + 1) * 512],
                            start=(pr == 0), stop=(pr == NP - 1),
                            perf_mode=DR)
                for t in ts_:
                    hi = ah_t[t // 2][:, t % 2, nb * 512:(nb + 1) * 512]
                    lo = al_t[t // 2][:, t % 2, nb * 512:(nb + 1) * 512]
                    nc.scalar.copy(hi, psa[t][:])
                    nc.vector.tensor_sub(lo, psa[t][:], hi)

            a_wave(0, range(8))
            a_wave(1, range(0, 4))
            a_wave(1, range(4, 8))

            # ====== phase Q: psq = 16*(A^T Xq) via 3-pass DR;
            # qt8 = (psq + 16*g)/32768 ======
            def q_wave(nb, ms_):
                psq = {m: psp.tile([128, 512], F32, tag=f"ps{m}",
                                   name=f"psQ{nb}_{m}") for m in ms_}
                combos = [(ah_t, xqh_t), (al_t, xqh_t), (ah_t, xql_t)]
                for tp in range(NP):
                    for ci, (a_src, x_src) in enumerate(combos):
                        for m in ms_:
                            nc.tensor.matmul(
                                psq[m][:],
                                a_src[tp][:, :, m * 128:(m + 1) * 128],
                                x_src[:, tp, :, nb * 512:(nb + 1) * 512],
                                start=(tp == 0 and ci == 0),
                                stop=(tp == NP - 1 and ci == 2),
                                perf_mode=DR)
                for m in ms_:
                    dst = qt8_t[m // 2][:, m % 2, nb * 512:(nb + 1) * 512]
                    if m % 2 == 0:
                        nc.vector.tensor_scalar(
                            dst, psq[m][:], g16_t[:, m:m + 1], 1.0 / 32768.0,
                            mybir.AluOpType.add, mybir.AluOpType.mult)
                    else:
                        nc.scalar.activation(
                            dst, psq[m][:], AF.Identity,
                            bias=g2_t[:, m:m + 1], scale=1.0 / 32768.0)

            q_wave(0, range(8))
            q_wave(1, range(0, 4))
            q_wave(1, range(4, 8))

        # ====== phase D: attention, blocked over s_q; blk0/blk1 scores
        # interleave ahead of Z^T so exp hi/lo drains stay off PE's path ==
        with tc.tile_pool(name="exsp", bufs=2) as exsp, \
             tc.tile_pool(name="ehlp", bufs=2) as ehlp, \
             tc.tile_pool(name="zhlp", bufs=2) as zhlp, \
             tc.tile_pool(name="otp", bufs=1) as otp, \
             tc.tile_pool(name="srp", bufs=2) as srp, \
             tc.tile_pool(name="rcp", bufs=2) as rcp:
            eh_t = {(blk, tp): ehlp.tile([128, 2, BQ], FP8,
                                         tag=f"eh{tp}", name=f"eh{blk}_{tp}")
                    for blk in range(NBLK) for tp in range(SP8)}
            el_t = {(blk, tp): ehlp.tile([128, 2, BQ], FP8,
                                         tag=f"el{tp}", name=f"el{blk}_{tp}")
                    for blk in range(NBLK) for tp in range(SP8)}
            zh_t = {(blk, zp): zhlp.tile([128, 2, BQ], FP8,
                                         tag=f"zh{zp}", name=f"zh{blk}_{zp}")
                    for blk in range(NBLK) for zp in range(NP)}
            zl_t = {(blk, zp): zhlp.tile([128, 2, BQ], FP8,
                                         tag=f"zl{zp}", name=f"zl{blk}_{zp}")
                    for blk in range(NBLK) for zp in range(NP)}

            def exp_drain(blk, m, pss):
                ex = exsp.tile([128, BQ], F32, tag=f"ex{m % 8}",
                               name=f"ex{blk}_{m}")
                nc.scalar.activation(ex[:], pss[:], AF.Exp,
                                     scale=INV_SCALE / 4.0)
                hi = eh_t[(blk, m // 2)][:, m % 2, :]
                lo = el_t[(blk, m // 2)][:, m % 2, :]
                nc.vector.tensor_copy(hi, ex[:])
                # alternate the residual subtract across DVE/Pool so neither
                # serial stream paces the exp-consuming matmuls
                if m % 2 == 0:
                    nc.vector.tensor_sub(lo, ex[:], hi)
                else:
                    nc.gpsimd.tensor_sub(lo, ex[:], hi)

            # scores blk0: k-outer 8-bank waves (start as xk8[0] lands)
            q0 = 0
            for half in range(2):
                ms = range(half * 8, half * 8 + 8)
                pss = {m: psp.tile([128, BQ], F32, tag=f"ps{m % 8}",
                                   name=f"psS0_{m}") for m in ms}
                for pr in range(NP):
                    for m in ms:
                        nc.tensor.matmul(
                            pss[m][:],
                            xk8_t[:, pr, :, m * 128:(m + 1) * 128],
                            qt8_t[pr][:, :, q0:q0 + BQ],
                            start=(pr == 0), stop=(pr == NP - 1),
                            perf_mode=DR)
                for m in ms:
                    exp_drain(0, m, pss[m])

            # scores blk1: 2-bank ping-pong
            q0 = BQ
            for m in range(MK):
                ps = psp.tile([128, BQ], F32, tag=f"ps{m % 2}",
                              name=f"psS1_{m}")
                for pr in range(NP):
                    nc.tensor.matmul(
                        ps[:],
                        xk8_t[:, pr, :, m * 128:(m + 1) * 128],
                        qt8_t[pr][:, :, q0:q0 + BQ],
                        start=(pr == 0), stop=(pr == NP - 1),
                        perf_mode=DR)
                exp_drain(1, m, ps)

            def zt_phase(blk):
                # Z^T/2 [e, q] = sum_s (Xv/2)^T exp: 3-pass DR, e_-outer.
                # eh-only passes run first so the el drains (one extra hop
                # behind eh) have the whole first loop to land.
                for e_ in range(EK):
                    ps = psp.tile([128, BQ], F32, tag=f"ps{2 + e_ % 2}",
                                  name=f"psZ{blk}_{e_}")
                    sl = slice(e_ * 128, (e_ + 1) * 128)
                    for tp in range(SP8):
                        nc.tensor.matmul(
                            ps[:], xvh_t[:, tp, :, sl], eh_t[(blk, tp)][:],
                            start=(tp == 0), stop=False, perf_mode=DR)
                        nc.tensor.matmul(
                            ps[:], xvl_t[:, tp, :, sl], eh_t[(blk, tp)][:],
                            start=False, stop=False, perf_mode=DR)
                    for tp in range(SP8):
                        nc.tensor.matmul(
                            ps[:], xvh_t[:, tp, :, sl], el_t[(blk, tp)][:],
                            start=False, stop=(tp == SP8 - 1), perf_mode=DR)
                    hi = zh_t[(blk, e_ // 2)][:, e_ % 2, :]
                    lo = zl_t[(blk, e_ // 2)][:, e_ % 2, :]
                    nc.scalar.copy(hi, ps[:])
                    nc.vector.tensor_sub(lo, ps[:], hi)

            def sums_phase(blk):
                # sums row = 16 * colsum(exp) via ones(16) DR matmuls,
                # then K=1 transpose to per-partition recip inputs
                # el's contribution to sums is +-2%/sqrt(2048) ~ 4e-4: skip it
                ps_row = psp.tile([128, BQ], F32, tag="ps6",
                                  name=f"psRow{blk}")
                for tp in range(SP8):
                    nc.tensor.matmul(ps_row[:], ones8[:],
                                     eh_t[(blk, tp)][:],
                                     start=(tp == 0), stop=(tp == SP8 - 1),
                                     perf_mode=DR)
                # fp16 row of 16*sums/1024 (~40, exact to 5e-4); stationary
                # for both the recip transpose and the rank-1 bv fold
                sums_sb = srp.tile([1, BQ], FP16, tag="sums_sb",
                                   name=f"sums_sb{blk}")
                nc.scalar.activation(sums_sb[:], ps_row[0:1, :],
                                     AF.Copy, scale=1.0 / 1024.0)
                sums16 = sums_sb[0:1, BQ - 128:BQ] if blk == NBLK - 1 else None
                psT = psp.tile([128, 4], F32, tag="ps7", name=f"psT{blk}")
                recips = []
                for it in range(BQ // 128):
                    nc.tensor.matmul(psT[:, it:it + 1],
                                     sums_sb[0:1, it * 128:(it + 1) * 128],
                                     ones1[:], start=True, stop=True)
                for it in range(BQ // 128):
                    rc = rcp.tile([128, 1], F32, tag=f"rc{it}",
                                  name=f"rc{blk}_{it}")
                    nc.vector.reciprocal(rc[:], psT[:, it:it + 1])
                    recips.append(rc)
                return recips, sums16

            def o_phase(blk, recips, sums16):
                # psO = 16*O via 3-pass DR over zh/zl x wvh/wvl.  One merged
                # out-DMA per i-tile except the last two (those stream
                # per-chunk so the end-of-kernel DMA chain is short).
                q0 = blk * BQ
                for it in range(BQ // 128):
                    ot = otp.tile([128, E], F32, tag=f"ot{it}",
                                  name=f"ot{blk}_{it}")
                    tail = (blk == NBLK - 1 and it >= BQ // 128 - 2)
                    last_it = (blk == NBLK - 1 and it == BQ // 128 - 1)
                    # taper the very last i-tile so the end-of-kernel
                    # Act->descgen->DMA chain covers only 128 columns
                    widths = [512, 256, 128, 128] if last_it else [512, 512]
                    f0 = 0
                    for ci, cw in enumerate(widths):
                        final = last_it and ci == len(widths) - 1
                        ps = psp.tile([128, cw], F32, tag=f"ps{4 + ci % 2}",
                                      name=f"psO{blk}_{it}_{ci}")
                        isl = slice(it * 128, (it + 1) * 128)
                        combos = [(zh_t, wvh_t), (zh_t, wvl_t), (zl_t, wvh_t)]
                        for zp in range(NP):
                            for k, (z_src, w_src) in enumerate(combos):
                                nc.tensor.matmul(
                                    ps[:],
                                    z_src[(blk, zp)][:, :, isl],
                                    w_src[:, zp, :, f0:f0 + cw],
                                    start=(zp == 0 and k == 0),
                                    stop=(zp == NP - 1 and k == 2
                                          and not final),
                                    perf_mode=DR)
                        if final:
                            # psO += (16*sums/1024) * (1024*bv); after the
                            # recip scale this is exactly + bv
                            nc.tensor.matmul(
                                ps[:], sums16,
                                bv16_row[0:1, f0:f0 + cw],
                                start=False, stop=True)
                        nc.scalar.activation(
                            ot[:, f0:f0 + cw], ps[:],
                            AF.Copy, scale=recips[it][:])
                        if not final:
                            nc.vector.tensor_add(
                                ot[:, f0:f0 + cw],
                                ot[:, f0:f0 + cw],
                                bv_bc[:, f0:f0 + cw])
                        if tail:
                            nc.sync.dma_start(
                                out[q0 + it * 128:q0 + (it + 1) * 128,
                                    f0:f0 + cw],
                                ot[:, f0:f0 + cw])
                        f0 += cw
                    if not tail:
                        nc.sync.dma_start(
                            out[q0 + it * 128:q0 + (it + 1) * 128, :],
                            ot[:])

            zt_phase(0)
            rec0 = sums_phase(0)
            zt_phase(1)
            rec1 = sums_phase(1)
            o_phase(0, *rec0)
            o_phase(1, *rec1)

    nc.compile()
    return nc


def _get_nc():
    if "nc" not in _cached:
        _cached["nc"] = _build()
    return _cached["nc"]


def _split8(x, s):
    """fp8 hi/lo pair of s*x (hi + lo == s*x up to lo's own rounding)."""
    xs = np.asarray(x, np.float32) * s
    hi = xs.astype(ml_dtypes.float8_e4m3)
    lo = (xs - hi.astype(np.float32)).astype(ml_dtypes.float8_e4m3)
    return hi, lo


def _pack_pairs(a):
    """[R, C] fp8 -> DoubleRow pair tiles, partition-major [128, nt*2*C]."""
    r, c = a.shape
    return np.ascontiguousarray(
        a.reshape(r // 256, 2, 128, c).transpose(2, 0, 1, 3)
        .reshape(128, (r // 128) * c))


def kernel(query, key, value, Wq, bq, Wk, bk, Wv, bv, **kw):
    query = np.asarray(query, dtype=np.float32)
    key = np.asarray(key, dtype=np.float32)
    value = np.asarray(value, dtype=np.float32)
    Wq = np.asarray(Wq, dtype=np.float32)
    Wk = np.asarray(Wk, dtype=np.float32)
    Wv = np.asarray(Wv, dtype=np.float32)
    bq = np.asarray(bq, dtype=np.float32)
    bv = np.asarray(bv, dtype=np.float32)

    def _f8_one(x, s):
        return _pack_pairs((np.asarray(x, np.float32) * s)
                           .astype(ml_dtypes.float8_e4m3))

    wq8_h = _f8_one(Wq.T, 32.0)
    wk8_h = _f8_one(Wk.T, 32.0)
    wvh, wvl = _split8(Wv, 32.0)
    wvh_h, wvl_h = _pack_pairs(wvh), _pack_pairs(wvl)

    g_dev = (Wk @ bq) * 1024.0        # [E]; bk cancels in softmax
    g16_h = np.ascontiguousarray(
        (16.0 * g_dev).reshape(EK, 128).T).astype(np.float32)
    gh2_h = np.ascontiguousarray(
        (g_dev / 2048.0).reshape(EK, 128).T).astype(np.float32)
    bv_h = np.ascontiguousarray(bv.reshape(1, E))
    bv16_h = np.ascontiguousarray(
        (1024.0 * bv).reshape(1, E)).astype(np.float16)

    key8 = {}
    val8 = {}
    for b in range(B):
        key8[b] = _f8_one(key[b].T, 8.0)
        vh, vl = _split8(value[b], 0.5)
        val8[b] = (_pack_pairs(vh), _pack_pairs(vl))

    in_maps = []
    for c in range(N_CORES):
        b, h = divmod(c, 2)
        xqh, xql = _split8(query[b, h * SQ:(h + 1) * SQ, :].T, 16.0)
        in_maps.append({
            "wq8": wq8_h, "wk8": wk8_h,
            "xq8h": _pack_pairs(xqh), "xq8l": _pack_pairs(xql),
            "xk8": key8[b], "xv8h": val8[b][0], "xv8l": val8[b][1],
            "wv8h": wvh_h, "wv8l": wvl_h,
            "g16h": g16_h, "gh2": gh2_h, "bvh": bv_h, "bv16h": bv16_h,
        })

    nc = _get_nc()
    res = bass_utils.run_bass_kernel_spmd(
        nc, in_maps, core_ids=list(range(N_CORES)), **kw)

    full = np.empty((B, S, E), dtype=np.float32)
    for c in range(N_CORES):
        b, h = divmod(c, 2)
        full[b, h * SQ:(h + 1) * SQ, :] = res.results[c]["out"]
    kernel.last_results = res
    return full


# revision 31
# speedup vs baseline: 1.0986x; 1.0049x over previous
"""Trainium2 Bass kernel for single-head attention model (v4: 3-pass fp8).

Reference computation (B=4, S=2048, E=1024, fp32):
    q = query @ Wq + bq;  k = key @ Wk + bk;  v = value @ Wv + bv
    scores = (q @ k^T) / sqrt(E)
    out = softmax(scores, axis=-1) @ v

Sharding: 8 cores; core c handles batch b = c // 2, query-row half
h = c % 2 (1024 q-rows). No collectives.

Algebraic restructure:  scores_ij = x^q_i A x^k_j + g.x^k_j with
A = Wq Wk^T (computed on device in fp8 DoubleRow), g = Wk bq (host);
bk cancels in softmax.  out = (attn @ Xv) @ Wv + bv.

All five matmul groups run fp8e4m3 DoubleRow (0.5 cyc/row at 256-deep
contraction = 4x bf16 MACs/cycle).  The bf16 phases of the previous
version (Q, Z^T, O) instead use a 3-pass residual expansion:
x ~= xh + xl with xh = fp8(s*x), xl = fp8(s*x - xh), and
(ah+al)(bh+bl) ~= ah*bh + ah*bl + al*bh  (al*bl ~ 4e-4, dropped).
All three passes share one PSUM accumulation group (same binary
scale), so each 3-pass group costs 0.75x the bf16 cycles with better
than bf16 accuracy.  Residual splits of device-produced tensors
(A, exp, Z) are drained as: hi = Act fp8 cast of PSUM, lo = DVE
tensor_sub(PSUM, hi).  Xq/Xv/Wv hi+lo pairs are packed on host.

Softmax sums come from fp8 ones(16.0)-row DoubleRow matmuls over the
eh/el tiles (PE, ~0 cost) instead of a DVE add chain, then a K=1
transpose matmul turns the [1,BQ] sums row into per-partition [128,1]
reciprocal inputs.  recip = 1/(16*sums) so the final Act scale folds
the /16 of psO = 16*O.  The last O chunk folds bv into PSUM via a
rank-1 sums_row x bv matmul; other chunks add a broadcast bv on DVE.

Error sources: one-pass fp8 A (wq8*wk8), k8, qt8 -> model rel-err
1.67e-2 (= previous version), PE busy ~105us vs 133us.
"""

import sys

sys.path.insert(0, "/opt/trn_rl_repo")

from contextlib import ExitStack

import ml_dtypes
import numpy as np

import concourse.mybir as mybir
import concourse.tile as tile
from concourse import bacc, bass_utils

BF16 = mybir.dt.bfloat16
FP16 = mybir.dt.float16
F32 = mybir.dt.float32
FP8 = mybir.dt.float8e4
F32R = mybir.dt.float32r
AF = mybir.ActivationFunctionType
DR = mybir.MatmulPerfMode.DoubleRow

B, S, E = 4, 2048, 1024
N_CORES = 8
SQ = S // 2          # q rows per core
BQ = 512             # s_q block width in attention phase
NBLK = SQ // BQ      # 2 blocks
EK = E // 128        # 8 tiles over e/a/c dims
MK = S // 128        # 16 s_k tiles
NP = EK // 2         # 4 pair-tiles over e contraction
SP8 = MK // 2        # 8 pair-tiles over s_k contraction
INV_SCALE = 1.0 / float(np.sqrt(E))

_cached = {}


def _build():
    nc = bacc.Bacc("TRN2", target_bir_lowering=False, debug=False,
                   num_devices=N_CORES)

    # host pre-packed fp8 DoubleRow inputs, partition-major [128, nt*2*cols]
    # (one row per partition => 1 descriptor/partition/DMA, few big DMAs)
    wq8 = nc.dram_tensor("wq8", [128, NP * 2 * E], FP8,
                         kind="ExternalInput").ap()
    wk8 = nc.dram_tensor("wk8", [128, NP * 2 * E], FP8,
                         kind="ExternalInput").ap()
    xq8h = nc.dram_tensor("xq8h", [128, NP * 2 * SQ], FP8,
                          kind="ExternalInput").ap()
    xq8l = nc.dram_tensor("xq8l", [128, NP * 2 * SQ], FP8,
                          kind="ExternalInput").ap()
    xk8 = nc.dram_tensor("xk8", [128, NP * 2 * S], FP8,
                         kind="ExternalInput").ap()
    xv8h = nc.dram_tensor("xv8h", [128, (S // 256) * 2 * E], FP8,
                          kind="ExternalInput").ap()
    xv8l = nc.dram_tensor("xv8l", [128, (S // 256) * 2 * E], FP8,
                          kind="ExternalInput").ap()
    wv8h = nc.dram_tensor("wv8h", [128, NP * 2 * E], FP8,
                          kind="ExternalInput").ap()
    wv8l = nc.dram_tensor("wv8l", [128, NP * 2 * E], FP8,
                          kind="ExternalInput").ap()
    # g = Wk @ bq arranged g_h[p, t] = g[t*128 + p]; two scale variants
    g16h = nc.dram_tensor("g16h", [128, EK], F32, kind="ExternalInput").ap()
    gh2 = nc.dram_tensor("gh2", [128, EK], F32, kind="ExternalInput").ap()
    bvh = nc.dram_tensor("bvh", [1, E], F32, kind="ExternalInput").ap()
    bv16h = nc.dram_tensor("bv16h", [1, E], FP16, kind="ExternalInput").ap()
    out = nc.dram_tensor("out", [SQ, E], F32, kind="ExternalOutput").ap()

    with tile.TileContext(nc) as tc, ExitStack() as top:
        # ---- long-lived pools ----
        consts = top.enter_context(tc.tile_pool(name="consts", bufs=1))
        qtpool = top.enter_context(tc.tile_pool(name="qtpool", bufs=1))
        xkpool = top.enter_context(tc.tile_pool(name="xkpool", bufs=1))
        xvpool = top.enter_context(tc.tile_pool(name="xvpool", bufs=1))
        wvpool = top.enter_context(tc.tile_pool(name="wvpool", bufs=1))

        # single shared PSUM pool: 8 tags x [128,512]f32 = 8 banks
        psp = top.enter_context(tc.tile_pool(name="psp", bufs=1, space="PSUM"))

        qt8_t = [qtpool.tile([128, 2, SQ], FP8, tag=f"qt8{p}", name=f"qt8{p}")
                 for p in range(NP)]
        xk8_t = xkpool.tile([128, NP, 2, S], FP8, tag="xk8", name="xk8")
        xvh_t = xvpool.tile([128, SP8, 2, E], FP8, tag="xvh", name="xvh")
        xvl_t = xvpool.tile([128, SP8, 2, E], FP8, tag="xvl", name="xvl")
        wvh_t = wvpool.tile([128, NP, 2, E], FP8, tag="wvh", name="wvh")
        wvl_t = wvpool.tile([128, NP, 2, E], FP8, tag="wvl", name="wvl")

        with tc.tile_pool(name="wqwk", bufs=1) as wqwkp, \
             tc.tile_pool(name="apool", bufs=1) as apool, \
             tc.tile_pool(name="xqpool", bufs=1) as xqpool:
            wq8_t = wqwkp.tile([128, NP, 2, E], FP8, tag="wq8", name="wq8")
            wk8_t = wqwkp.tile([128, NP, 2, E], FP8, tag="wk8", name="wk8")
            ah_t = [apool.tile([128, 2, E], FP8, tag=f"ah{t}", name=f"ah{t}")
                    for t in range(NP)]
            al_t = [apool.tile([128, 2, E], FP8, tag=f"al{t}", name=f"al{t}")
                    for t in range(NP)]
            xqh_t = xqpool.tile([128, NP, 2, SQ], FP8, tag="xqh", name="xqh")
            xql_t = xqpool.tile([128, NP, 2, SQ], FP8, tag="xql", name="xql")

            ones_r = consts.tile([128, 256], F32)
            ones_f32r = ones_r[:].bitcast(F32R)
            ones8 = consts.tile([128, 2, 128], FP8)  # value 16.0 (exact fp8)
            ones1 = consts.tile([1, 1], FP16)       # moving for K=1 transpose

            # ---- PE warm-up through the DMA lead-in (reads ones_r
            # uninitialized on purpose; values never consumed) ----
            warm = psp.tile([128, 256], F32, tag="ps0", name="warm")
            for _ in range(28):
                nc.tensor.matmul(warm[:], ones_f32r[:, 0:128],
                                 ones_f32r, start=True, stop=True)
            nc.vector.memset(ones_r[:], 1.0)
            nc.vector.memset(ones8[:], 16.0)
            # psT = (16*sums/1024) * 1024 = 16*sums -> recip = 1/(16*sums)
            nc.vector.memset(ones1[:], 1024.0)

            # ---- DMA issue order = consumption order; big partition-major
            # transfers (1 descriptor/partition), split only for pipelining
            half_q = NP // 2 * 2 * E
            for hh in range(2):
                nc.sync.dma_start(wq8_t[:, hh * 2:(hh + 1) * 2, :, :],
                                  wq8[:, hh * half_q:(hh + 1) * half_q])
                nc.sync.dma_start(wk8_t[:, hh * 2:(hh + 1) * 2, :, :],
                                  wk8[:, hh * half_q:(hh + 1) * half_q])
            g16_t = consts.tile([128, EK], F32)
            nc.sync.dma_start(g16_t[:], g16h)
            g2_t = consts.tile([128, EK], F32)
            nc.sync.dma_start(g2_t[:], gh2)
            bv_row = consts.tile([1, E], F32)
            nc.sync.dma_start(bv_row[:], bvh)
            bv16_row = consts.tile([1, E], FP16)
            nc.sync.dma_start(bv16_row[:], bv16h)
            bv_bc = consts.tile([128, E], F32)
            nc.gpsimd.partition_broadcast(bv_bc[:], bv_row[:])
            nc.sync.dma_start(xqh_t[:], xq8h)
            nc.sync.dma_start(xql_t[:], xq8l)
            half_k = NP // 2 * 2 * S
            for hh in range(2):
                nc.sync.dma_start(xk8_t[:, hh * 2:(hh + 1) * 2, :, :],
                                  xk8[:, hh * half_k:(hh + 1) * half_k])
            half_v = SP8 // 2 * 2 * E
            for hh in range(2):
                nc.sync.dma_start(xvh_t[:, hh * 4:(hh + 1) * 4, :, :],
                                  xv8h[:, hh * half_v:(hh + 1) * half_v])
            for hh in range(2):
                nc.sync.dma_start(xvl_t[:, hh * 4:(hh + 1) * 4, :, :],
                                  xv8l[:, hh * half_v:(hh + 1) * half_v])
            nc.sync.dma_start(wvh_t[:], wv8h)
            nc.sync.dma_start(wvl_t[:], wv8l)

            # ====== phase A: psa = Wq Wk^T (x1024), 1-pass fp8 DR;
            # drains split hi/lo: ah = fp8(psa) [Act], al = psa-ah [DVE] ====
            def a_wave(nb, ts_):
                psa = {t: psp.tile([128, 512], F32, tag=f"ps{t}",
                                   name=f"psA{nb}_{t}") for t in ts_}
                for pr in range(NP):
                    for t in ts_:
                        nc.tensor.matmul(
                            psa[t][:],
                            wq8_t[:, pr, :, t * 128:(t + 1) * 128],
                            wk8_t[:, pr, :, nb * 512:(nb + 1) * 512],
                            start=(pr == 0), stop=(pr == NP - 1),
                            perf_mode=DR)
                for t in ts_:
                    hi = ah_t[t // 2][:, t % 2, nb * 512:(nb + 1) * 512]
                    lo = al_t[t // 2][:, t % 2, nb * 512:(nb + 1) * 512]
                    nc.scalar.copy(hi, psa[t][:])
                    nc.vector.tensor_sub(lo, psa[t][:], hi)

            a_wave(0, range(8))
            a_wave(1, range(0, 4))
            a_wave(1, range(4, 8))

            # ====== phase Q: psq = 16*(A^T Xq) via 3-pass DR;
            # qt8 = (psq + 16*g)/32768 ======
            def q_wave(nb, ms_):
                psq = {m: psp.tile([128, 512], F32, tag=f"ps{m}",
                                   name=f"psQ{nb}_{m}") for m in ms_}
                combos = [(ah_t, xqh_t), (al_t, xqh_t), (ah_t, xql_t)]
                for tp in range(NP):
                    for ci, (a_src, x_src) in enumerate(combos):
                        for m in ms_:
                            nc.tensor.matmul(
                                psq[m][:],
                                a_src[tp][:, :, m * 128:(m + 1) * 128],
                                x_src[:, tp, :, nb * 512:(nb + 1) * 512],
                                start=(tp == 0 and ci == 0),
                                stop=(tp == NP - 1 and ci == 2),
                                perf_mode=DR)
                for m in ms_:
                    dst = qt8_t[m // 2][:, m % 2, nb * 512:(nb + 1) * 512]
                    if m % 2 == 0:
                        nc.vector.tensor_scalar(
                            dst, psq[m][:], g16_t[:, m:m + 1], 1.0 / 32768.0,
                            mybir.AluOpType.add, mybir.AluOpType.mult)
                    else:
                        nc.scalar.activation(
                            dst, psq[m][:], AF.Identity,
                            bias=g2_t[:, m:m + 1], scale=1.0 / 32768.0)

            q_wave(0, range(8))
            q_wave(1, range(0, 4))
            q_wave(1, range(4, 8))

        # ====== phase D: attention, blocked over s_q; blk0/blk1 scores
        # interleave ahead of Z^T so exp hi/lo drains stay off PE's path ==
        with tc.tile_pool(name="exsp", bufs=2) as exsp, \
             tc.tile_pool(name="ehlp", bufs=2) as ehlp, \
             tc.tile_pool(name="zhlp", bufs=2) as zhlp, \
             tc.tile_pool(name="otp", bufs=1) as otp, \
             tc.tile_pool(name="srp", bufs=2) as srp, \
             tc.tile_pool(name="rcp", bufs=2) as rcp:
            eh_t = {(blk, tp): ehlp.tile([128, 2, BQ], FP8,
                                         tag=f"eh{tp}", name=f"eh{blk}_{tp}")
                    for blk in range(NBLK) for tp in range(SP8)}
            el_t = {(blk, tp): ehlp.tile([128, 2, BQ], FP8,
                                         tag=f"el{tp}", name=f"el{blk}_{tp}")
                    for blk in range(NBLK) for tp in range(SP8)}
            zh_t = {(blk, zp): zhlp.tile([128, 2, BQ], FP8,
                                         tag=f"zh{zp}", name=f"zh{blk}_{zp}")
                    for blk in range(NBLK) for zp in range(NP)}
            zl_t = {(blk, zp): zhlp.tile([128, 2, BQ], FP8,
                                         tag=f"zl{zp}", name=f"zl{blk}_{zp}")
                    for blk in range(NBLK) for zp in range(NP)}

            def exp_drain(blk, m, pss):
                ex = exsp.tile([128, BQ], F32, tag=f"ex{m % 8}",
                               name=f"ex{blk}_{m}")
                nc.scalar.activation(ex[:], pss[:], AF.Exp,
                                     scale=INV_SCALE / 4.0)
                hi = eh_t[(blk, m // 2)][:, m % 2, :]
                lo = el_t[(blk, m // 2)][:, m % 2, :]
                nc.vector.tensor_copy(hi, ex[:])
                # alternate the residual subtract across DVE/Pool so neither
                # serial stream paces the exp-consuming matmuls
                if m % 2 == 0:
                    nc.vector.tensor_sub(lo, ex[:], hi)
                else:
                    nc.gpsimd.tensor_sub(lo, ex[:], hi)

            # scores blk0: k-outer 8-bank waves (start as xk8[0] lands)
            q0 = 0
            for half in range(2):
                ms = range(half * 8, half * 8 + 8)
                pss = {m: psp.tile([128, BQ], F32, tag=f"ps{m % 8}",
                                   name=f"psS0_{m}") for m in ms}
                for pr in range(NP):
                    for m in ms:
                        nc.tensor.matmul(
                            pss[m][:],
                            xk8_t[:, pr, :, m * 128:(m + 1) * 128],
                            qt8_t[pr][:, :, q0:q0 + BQ],
                            start=(pr == 0), stop=(pr == NP - 1),
                            perf_mode=DR)
                for m in ms:
                    exp_drain(0, m, pss[m])

            # scores blk1: 2-bank ping-pong
            q0 = BQ
            for m in range(MK):
                ps = psp.tile([128, BQ], F32, tag=f"ps{m % 2}",
                              name=f"psS1_{m}")
                for pr in range(NP):
                    nc.tensor.matmul(
                        ps[:],
                        xk8_t[:, pr, :, m * 128:(m + 1) * 128],
                        qt8_t[pr][:, :, q0:q0 + BQ],
                        start=(pr == 0), stop=(pr == NP - 1),
                        perf_mode=DR)
                exp_drain(1, m, ps)

            def zt_phase(blk):
                # Z^T/2 [e, q] = sum_s (Xv/2)^T exp: 3-pass DR, e_-outer.
                # eh-only passes run first so the el drains (one extra hop
                # behind eh) have the whole first loop to land.
                for e_ in range(EK):
                    ps = psp.tile([128, BQ], F32, tag=f"ps{2 + e_ % 2}",
                                  name=f"psZ{blk}_{e_}")
                    sl = slice(e_ * 128, (e_ + 1) * 128)
                    for tp in range(SP8):
                        nc.tensor.matmul(
                            ps[:], xvh_t[:, tp, :, sl], eh_t[(blk, tp)][:],
                            start=(tp == 0), stop=False, perf_mode=DR)
                        nc.tensor.matmul(
                            ps[:], xvl_t[:, tp, :, sl], eh_t[(blk, tp)][:],
                            start=False, stop=False, perf_mode=DR)
                    for tp in range(SP8):
                        nc.tensor.matmul(
                            ps[:], xvh_t[:, tp, :, sl], el_t[(blk, tp)][:],
                            start=False, stop=(tp == SP8 - 1), perf_mode=DR)
                    hi = zh_t[(blk, e_ // 2)][:, e_ % 2, :]
                    lo = zl_t[(blk, e_ // 2)][:, e_ % 2, :]
                    nc.scalar.copy(hi, ps[:])
                    nc.vector.tensor_sub(lo, ps[:], hi)

            def sums_phase(blk):
                # sums row = 16 * colsum(exp) via ones(16) DR matmuls,
                # then K=1 transpose to per-partition recip inputs
                # el's contribution to sums is +-2%/sqrt(2048) ~ 4e-4: skip it
                ps_row = psp.tile([128, BQ], F32, tag="ps6",
                                  name=f"psRow{blk}")
                for tp in range(SP8):
                    nc.tensor.matmul(ps_row[:], ones8[:],
                                     eh_t[(blk, tp)][:],
                                     start=(tp == 0), stop=(tp == SP8 - 1),
                                     perf_mode=DR)
                # fp16 row of 16*sums/1024 (~40, exact to 5e-4); stationary
                # for both the recip transpose and the rank-1 bv fold
                sums_sb = srp.tile([1, BQ], FP16, tag="sums_sb",
                                   name=f"sums_sb{blk}")
                nc.scalar.activation(sums_sb[:], ps_row[0:1, :],
                                     AF.Copy, scale=1.0 / 1024.0)
                sums16 = sums_sb[0:1, BQ - 128:BQ] if blk == NBLK - 1 else None
                psT = psp.tile([128, 4], F32, tag="ps7", name=f"psT{blk}")
                recips = []
                for it in range(BQ // 128):
                    nc.tensor.matmul(psT[:, it:it + 1],
                                     sums_sb[0:1, it * 128:(it + 1) * 128],
                                     ones1[:], start=True, stop=True)
                for it in range(BQ // 128):
                    rc = rcp.tile([128, 1], F32, tag=f"rc{it}",
                                  name=f"rc{blk}_{it}")
                    nc.vector.reciprocal(rc[:], psT[:, it:it + 1])
                    recips.append(rc)
                return recips, sums16

            def o_phase(blk, recips, sums16):
                # psO = 16*O via 3-pass DR over zh/zl x wvh/wvl.  One merged
                # out-DMA per i-tile except the last two (those stream
                # per-chunk so the end-of-kernel DMA chain is short).
                q0 = blk * BQ
                for it in range(BQ // 128):
                    ot = otp.tile([128, E], F32, tag=f"ot{it}",
                                  name=f"ot{blk}_{it}")
                    tail = (blk == NBLK - 1 and it >= BQ // 128 - 2)
                    last_it = (blk == NBLK - 1 and it == BQ // 128 - 1)
                    # taper the very last i-tile so the end-of-kernel
                    # Act->descgen->DMA chain covers only 128 columns
                    widths = [512, 384, 128] if last_it else [512, 512]
                    f0 = 0
                    for ci, cw in enumerate(widths):
                        final = last_it and ci == len(widths) - 1
                        ps = psp.tile([128, cw], F32, tag=f"ps{4 + ci % 2}",
                                      name=f"psO{blk}_{it}_{ci}")
                        isl = slice(it * 128, (it + 1) * 128)
                        combos = [(zh_t, wvh_t), (zh_t, wvl_t), (zl_t, wvh_t)]
                        for zp in range(NP):
                            for k, (z_src, w_src) in enumerate(combos):
                                nc.tensor.matmul(
                                    ps[:],
                                    z_src[(blk, zp)][:, :, isl],
                                    w_src[:, zp, :, f0:f0 + cw],
                                    start=(zp == 0 and k == 0),
                                    stop=(zp == NP - 1 and k == 2
                                          and not final),
                                    perf_mode=DR)
                        if final:
                            # psO += (16*sums/1024) * (1024*bv); after the
                            # recip scale this is exactly + bv
                            nc.tensor.matmul(
                                ps[:], sums16,
                                bv16_row[0:1, f0:f0 + cw],
                                start=False, stop=True)
                        nc.scalar.activation(
                            ot[:, f0:f0 + cw], ps[:],
                            AF.Copy, scale=recips[it][:])
                        if not final:
                            nc.vector.tensor_add(
                                ot[:, f0:f0 + cw],
                                ot[:, f0:f0 + cw],
                                bv_bc[:, f0:f0 + cw])
                        if tail:
                            nc.sync.dma_start(
                                out[q0 + it * 128:q0 + (it + 1) * 128,
                                    f0:f0 + cw],
                                ot[:, f0:f0 + cw])
                        f0 += cw
                    if not tail:
                        nc.sync.dma_start(
                            out[q0 + it * 128:q0 + (it + 1) * 128, :],
                            ot[:])

            zt_phase(0)
            rec0 = sums_phase(0)
            zt_phase(1)
            rec1 = sums_phase(1)
            o_phase(0, *rec0)
            o_phase(1, *rec1)

    nc.compile()
    return nc


def _get_nc():
    if "nc" not in _cached:
        _cached["nc"] = _build()
    return _cached["nc"]


def _split8(x, s):
    """fp8 hi/lo pair of s*x (hi + lo == s*x up to lo's own rounding)."""
    xs = np.asarray(x, np.float32) * s
    hi = xs.astype(ml_dtypes.float8_e4m3)
    lo = (xs - hi.astype(np.float32)).astype(ml_dtypes.float8_e4m3)
    return hi, lo


def _pack_pairs(a):
    """[R, C] fp8 -> DoubleRow pair tiles, partition-major [128, nt*2*C]."""
    r, c = a.shape
    return np.ascontiguousarray(
        a.reshape(r // 256, 2, 128, c).transpose(2, 0, 1, 3)
        .reshape(128, (r // 128) * c))


def kernel(query, key, value, Wq, bq, Wk, bk, Wv, bv, **kw):
    query = np.asarray(query, dtype=np.float32)
    key = np.asarray(key, dtype=np.float32)
    value = np.asarray(value, dtype=np.float32)
    Wq = np.asarray(Wq, dtype=np.float32)
    Wk = np.asarray(Wk, dtype=np.float32)
    Wv = np.asarray(Wv, dtype=np.float32)
    bq = np.asarray(bq, dtype=np.float32)
    bv = np.asarray(bv, dtype=np.float32)

    def _f8_one(x, s):
        return _pack_pairs((np.asarray(x, np.float32) * s)
                           .astype(ml_dtypes.float8_e4m3))

    wq8_h = _f8_one(Wq.T, 32.0)
    wk8_h = _f8_one(Wk.T, 32.0)
    wvh, wvl = _split8(Wv, 32.0)
    wvh_h, wvl_h = _pack_pairs(wvh), _pack_pairs(wvl)

    g_dev = (Wk @ bq) * 1024.0        # [E]; bk cancels in softmax
    g16_h = np.ascontiguousarray(
        (16.0 * g_dev).reshape(EK, 128).T).astype(np.float32)
    gh2_h = np.ascontiguousarray(
        (g_dev / 2048.0).reshape(EK, 128).T).astype(np.float32)
    bv_h = np.ascontiguousarray(bv.reshape(1, E))
    bv16_h = np.ascontiguousarray(
        (1024.0 * bv).reshape(1, E)).astype(np.float16)

    key8 = {}
    val8 = {}
    for b in range(B):
        key8[b] = _f8_one(key[b].T, 8.0)
        vh, vl = _split8(value[b], 0.5)
        val8[b] = (_pack_pairs(vh), _pack_pairs(vl))

    in_maps = []
    for c in range(N_CORES):
        b, h = divmod(c, 2)
        xqh, xql = _split8(query[b, h * SQ:(h + 1) * SQ, :].T, 16.0)
        in_maps.append({
            "wq8": wq8_h, "wk8": wk8_h,
            "xq8h": _pack_pairs(xqh), "xq8l": _pack_pairs(xql),
            "xk8": key8[b], "xv8h": val8[b][0], "xv8l": val8[b][1],
            "wv8h": wvh_h, "wv8l": wvl_h,
            "g16h": g16_h, "gh2": gh2_h, "bvh": bv_h, "bv16h": bv16_h,
        })

    nc = _get_nc()
    res = bass_utils.run_bass_kernel_spmd(
        nc, in_maps, core_ids=list(range(N_CORES)), **kw)

    full = np.empty((B, S, E), dtype=np.float32)
    for c in range(N_CORES):
        b, h = divmod(c, 2)
        full[b, h * SQ:(h + 1) * SQ, :] = res.results[c]["out"]
    kernel.last_results = res
    return full


# revision 33
# speedup vs baseline: 1.1016x; 1.0027x over previous
"""Trainium2 Bass kernel for single-head attention model (v4: 3-pass fp8).

Reference computation (B=4, S=2048, E=1024, fp32):
    q = query @ Wq + bq;  k = key @ Wk + bk;  v = value @ Wv + bv
    scores = (q @ k^T) / sqrt(E)
    out = softmax(scores, axis=-1) @ v

Sharding: 8 cores; core c handles batch b = c // 2, query-row half
h = c % 2 (1024 q-rows). No collectives.

Algebraic restructure:  scores_ij = x^q_i A x^k_j + g.x^k_j with
A = Wq Wk^T (computed on device in fp8 DoubleRow), g = Wk bq (host);
bk cancels in softmax.  out = (attn @ Xv) @ Wv + bv.

All five matmul groups run fp8e4m3 DoubleRow (0.5 cyc/row at 256-deep
contraction = 4x bf16 MACs/cycle).  The bf16 phases of the previous
version (Q, Z^T, O) instead use a 3-pass residual expansion:
x ~= xh + xl with xh = fp8(s*x), xl = fp8(s*x - xh), and
(ah+al)(bh+bl) ~= ah*bh + ah*bl + al*bh  (al*bl ~ 4e-4, dropped).
All three passes share one PSUM accumulation group (same binary
scale), so each 3-pass group costs 0.75x the bf16 cycles with better
than bf16 accuracy.  Residual splits of device-produced tensors
(A, exp, Z) are drained as: hi = Act fp8 cast of PSUM, lo = DVE
tensor_sub(PSUM, hi).  Xq/Xv/Wv hi+lo pairs are packed on host.

Softmax sums come from fp8 ones(16.0)-row DoubleRow matmuls over the
eh/el tiles (PE, ~0 cost) instead of a DVE add chain, then a K=1
transpose matmul turns the [1,BQ] sums row into per-partition [128,1]
reciprocal inputs.  recip = 1/(16*sums) so the final Act scale folds
the /16 of psO = 16*O.  The last O chunk folds bv into PSUM via a
rank-1 sums_row x bv matmul; other chunks add a broadcast bv on DVE.

Error sources: one-pass fp8 A (wq8*wk8), k8, qt8 -> model rel-err
1.67e-2 (= previous version), PE busy ~105us vs 133us.
"""

import sys

sys.path.insert(0, "/opt/trn_rl_repo")

from contextlib import ExitStack

import ml_dtypes
import numpy as np

import concourse.mybir as mybir
import concourse.tile as tile
from concourse import bacc, bass_utils

BF16 = mybir.dt.bfloat16
FP16 = mybir.dt.float16
F32 = mybir.dt.float32
FP8 = mybir.dt.float8e4
F32R = mybir.dt.float32r
AF = mybir.ActivationFunctionType
DR = mybir.MatmulPerfMode.DoubleRow

B, S, E = 4, 2048, 1024
N_CORES = 8
SQ = S // 2          # q rows per core
BQ = 512             # s_q block width in attention phase
NBLK = SQ // BQ      # 2 blocks
EK = E // 128        # 8 tiles over e/a/c dims
MK = S // 128        # 16 s_k tiles
NP = EK // 2         # 4 pair-tiles over e contraction
SP8 = MK // 2        # 8 pair-tiles over s_k contraction
INV_SCALE = 1.0 / float(np.sqrt(E))

_cached = {}


def _build():
    nc = bacc.Bacc("TRN2", target_bir_lowering=False, debug=False,
                   num_devices=N_CORES)

    # host pre-packed fp8 DoubleRow inputs, partition-major [128, nt*2*cols]
    # (one row per partition => 1 descriptor/partition/DMA, few big DMAs)
    wq8 = nc.dram_tensor("wq8", [128, NP * 2 * E], FP8,
                         kind="ExternalInput").ap()
    wk8 = nc.dram_tensor("wk8", [128, NP * 2 * E], FP8,
                         kind="ExternalInput").ap()
    xq8h = nc.dram_tensor("xq8h", [128, NP * 2 * SQ], FP8,
                          kind="ExternalInput").ap()
    xq8l = nc.dram_tensor("xq8l", [128, NP * 2 * SQ], FP8,
                          kind="ExternalInput").ap()
    xk8 = nc.dram_tensor("xk8", [128, NP * 2 * S], FP8,
                         kind="ExternalInput").ap()
    xv8h = nc.dram_tensor("xv8h", [128, (S // 256) * 2 * E], FP8,
                          kind="ExternalInput").ap()
    xv8l = nc.dram_tensor("xv8l", [128, (S // 256) * 2 * E], FP8,
                          kind="ExternalInput").ap()
    wv8h = nc.dram_tensor("wv8h", [128, NP * 2 * E], FP8,
                          kind="ExternalInput").ap()
    wv8l = nc.dram_tensor("wv8l", [128, NP * 2 * E], FP8,
                          kind="ExternalInput").ap()
    # g = Wk @ bq arranged g_h[p, t] = g[t*128 + p]; two scale variants
    g16h = nc.dram_tensor("g16h", [128, EK], F32, kind="ExternalInput").ap()
    gh2 = nc.dram_tensor("gh2", [128, EK], F32, kind="ExternalInput").ap()
    bvh = nc.dram_tensor("bvh", [1, E], F32, kind="ExternalInput").ap()
    bv16h = nc.dram_tensor("bv16h", [1, E], FP16, kind="ExternalInput").ap()
    out = nc.dram_tensor("out", [SQ, E], F32, kind="ExternalOutput").ap()

    with tile.TileContext(nc) as tc, ExitStack() as top:
        # ---- long-lived pools ----
        consts = top.enter_context(tc.tile_pool(name="consts", bufs=1))
        qtpool = top.enter_context(tc.tile_pool(name="qtpool", bufs=1))
        xkpool = top.enter_context(tc.tile_pool(name="xkpool", bufs=1))
        xvpool = top.enter_context(tc.tile_pool(name="xvpool", bufs=1))
        wvpool = top.enter_context(tc.tile_pool(name="wvpool", bufs=1))

        # single shared PSUM pool: 8 tags x [128,512]f32 = 8 banks
        psp = top.enter_context(tc.tile_pool(name="psp", bufs=1, space="PSUM"))

        qt8_t = [qtpool.tile([128, 2, SQ], FP8, tag=f"qt8{p}", name=f"qt8{p}")
                 for p in range(NP)]
        xk8_t = xkpool.tile([128, NP, 2, S], FP8, tag="xk8", name="xk8")
        xvh_t = xvpool.tile([128, SP8, 2, E], FP8, tag="xvh", name="xvh")
        xvl_t = xvpool.tile([128, SP8, 2, E], FP8, tag="xvl", name="xvl")
        wvh_t = wvpool.tile([128, NP, 2, E], FP8, tag="wvh", name="wvh")
        wvl_t = wvpool.tile([128, NP, 2, E], FP8, tag="wvl", name="wvl")

        with tc.tile_pool(name="wqwk", bufs=1) as wqwkp, \
             tc.tile_pool(name="apool", bufs=1) as apool, \
             tc.tile_pool(name="xqpool", bufs=1) as xqpool:
            wq8_t = wqwkp.tile([128, NP, 2, E], FP8, tag="wq8", name="wq8")
            wk8_t = wqwkp.tile([128, NP, 2, E], FP8, tag="wk8", name="wk8")
            ah_t = [apool.tile([128, 2, E], FP8, tag=f"ah{t}", name=f"ah{t}")
                    for t in range(NP)]
            al_t = [apool.tile([128, 2, E], FP8, tag=f"al{t}", name=f"al{t}")
                    for t in range(NP)]
            xqh_t = xqpool.tile([128, NP, 2, SQ], FP8, tag="xqh", name="xqh")
            xql_t = xqpool.tile([128, NP, 2, SQ], FP8, tag="xql", name="xql")

            ones_r = consts.tile([128, 256], F32)
            ones_f32r = ones_r[:].bitcast(F32R)
            ones8 = consts.tile([128, 2, 128], FP8)  # value 16.0 (exact fp8)
            ones1 = consts.tile([1, 1], FP16)       # moving for K=1 transpose

            # ---- PE warm-up through the DMA lead-in (reads ones_r
            # uninitialized on purpose; values never consumed) ----
            warm = psp.tile([128, 256], F32, tag="ps0", name="warm")
            for _ in range(28):
                nc.tensor.matmul(warm[:], ones_f32r[:, 0:128],
                                 ones_f32r, start=True, stop=True)
            nc.vector.memset(ones_r[:], 1.0)
            nc.vector.memset(ones8[:], 16.0)
            # psT = (16*sums/1024) * 1024 = 16*sums -> recip = 1/(16*sums)
            nc.vector.memset(ones1[:], 1024.0)

            # ---- DMA issue order = consumption order; big partition-major
            # transfers (1 descriptor/partition), split only for pipelining
            half_q = NP // 2 * 2 * E
            for hh in range(2):
                nc.sync.dma_start(wq8_t[:, hh * 2:(hh + 1) * 2, :, :],
                                  wq8[:, hh * half_q:(hh + 1) * half_q])
                nc.sync.dma_start(wk8_t[:, hh * 2:(hh + 1) * 2, :, :],
                                  wk8[:, hh * half_q:(hh + 1) * half_q])
            g16_t = consts.tile([128, EK], F32)
            nc.sync.dma_start(g16_t[:], g16h)
            g2_t = consts.tile([128, EK], F32)
            nc.sync.dma_start(g2_t[:], gh2)
            bv_row = consts.tile([1, E], F32)
            nc.sync.dma_start(bv_row[:], bvh)
            bv16_row = consts.tile([1, E], FP16)
            nc.sync.dma_start(bv16_row[:], bv16h)
            bv_bc = consts.tile([128, E], F32)
            nc.gpsimd.partition_broadcast(bv_bc[:], bv_row[:])
            nc.sync.dma_start(xqh_t[:], xq8h)
            nc.sync.dma_start(xql_t[:], xq8l)
            half_k = NP // 2 * 2 * S
            for hh in range(2):
                nc.sync.dma_start(xk8_t[:, hh * 2:(hh + 1) * 2, :, :],
                                  xk8[:, hh * half_k:(hh + 1) * half_k])
            half_v = SP8 // 2 * 2 * E
            for hh in range(2):
                nc.sync.dma_start(xvh_t[:, hh * 4:(hh + 1) * 4, :, :],
                                  xv8h[:, hh * half_v:(hh + 1) * half_v])
            for hh in range(2):
                nc.sync.dma_start(xvl_t[:, hh * 4:(hh + 1) * 4, :, :],
                                  xv8l[:, hh * half_v:(hh + 1) * half_v])
            nc.sync.dma_start(wvh_t[:], wv8h)
            nc.sync.dma_start(wvl_t[:], wv8l)

            # ====== phase A: psa = Wq Wk^T (x1024), 1-pass fp8 DR;
            # drains split hi/lo: ah = fp8(psa) [Act], al = psa-ah [DVE] ====
            def a_wave(nb, ts_):
                psa = {t: psp.tile([128, 512], F32, tag=f"ps{t}",
                                   name=f"psA{nb}_{t}") for t in ts_}
                for pr in range(NP):
                    for t in ts_:
                        nc.tensor.matmul(
                            psa[t][:],
                            wq8_t[:, pr, :, t * 128:(t + 1) * 128],
                            wk8_t[:, pr, :, nb * 512:(nb + 1) * 512],
                            start=(pr == 0), stop=(pr == NP - 1),
                            perf_mode=DR)
                for t in ts_:
                    hi = ah_t[t // 2][:, t % 2, nb * 512:(nb + 1) * 512]
                    lo = al_t[t // 2][:, t % 2, nb * 512:(nb + 1) * 512]
                    nc.scalar.copy(hi, psa[t][:])
                    nc.vector.tensor_sub(lo, psa[t][:], hi)

            a_wave(0, range(8))
            a_wave(1, range(0, 4))
            a_wave(1, range(4, 8))

            # ====== phase Q: psq = 16*(A^T Xq) via 3-pass DR;
            # qt8 = (psq + 16*g)/32768 ======
            def q_wave(nb, ms_):
                # m-outer: each psq[m] completes (and drains) as soon as its
                # 12 passes finish, so the next wave's bank is free in time
                combos = [(ah_t, xqh_t), (ah_t, xql_t), (al_t, xqh_t)]
                for m in ms_:
                    psq = psp.tile([128, 512], F32, tag=f"ps{m}",
                                   name=f"psQ{nb}_{m}")
                    for tp in range(NP):
                        for ci, (a_src, x_src) in enumerate(combos):
                            nc.tensor.matmul(
                                psq[:],
                                a_src[tp][:, :, m * 128:(m + 1) * 128],
                                x_src[:, tp, :, nb * 512:(nb + 1) * 512],
                                start=(tp == 0 and ci == 0),
                                stop=(tp == NP - 1 and ci == 2),
                                perf_mode=DR)
                    dst = qt8_t[m // 2][:, m % 2, nb * 512:(nb + 1) * 512]
                    if m % 2 == 0:
                        nc.vector.tensor_scalar(
                            dst, psq[:], g16_t[:, m:m + 1], 1.0 / 32768.0,
                            mybir.AluOpType.add, mybir.AluOpType.mult)
                    else:
                        nc.scalar.activation(
                            dst, psq[:], AF.Identity,
                            bias=g2_t[:, m:m + 1], scale=1.0 / 32768.0)

            q_wave(0, range(8))
            q_wave(1, range(8))

        # ====== phase D: attention, blocked over s_q; blk0/blk1 scores
        # interleave ahead of Z^T so exp hi/lo drains stay off PE's path ==
        with tc.tile_pool(name="exsp", bufs=2) as exsp, \
             tc.tile_pool(name="ehlp", bufs=2) as ehlp, \
             tc.tile_pool(name="zhlp", bufs=2) as zhlp, \
             tc.tile_pool(name="otp", bufs=1) as otp, \
             tc.tile_pool(name="srp", bufs=2) as srp, \
             tc.tile_pool(name="rcp", bufs=2) as rcp:
            eh_t = {(blk, tp): ehlp.tile([128, 2, BQ], FP8,
                                         tag=f"eh{tp}", name=f"eh{blk}_{tp}")
                    for blk in range(NBLK) for tp in range(SP8)}
            el_t = {(blk, tp): ehlp.tile([128, 2, BQ], FP8,
                                         tag=f"el{tp}", name=f"el{blk}_{tp}")
                    for blk in range(NBLK) for tp in range(SP8)}
            zh_t = {(blk, zp): zhlp.tile([128, 2, BQ], FP8,
                                         tag=f"zh{zp}", name=f"zh{blk}_{zp}")
                    for blk in range(NBLK) for zp in range(NP)}
            zl_t = {(blk, zp): zhlp.tile([128, 2, BQ], FP8,
                                         tag=f"zl{zp}", name=f"zl{blk}_{zp}")
                    for blk in range(NBLK) for zp in range(NP)}

            def exp_drain(blk, m, pss):
                ex = exsp.tile([128, BQ], F32, tag=f"ex{m % 8}",
                               name=f"ex{blk}_{m}")
                nc.scalar.activation(ex[:], pss[:], AF.Exp,
                                     scale=INV_SCALE / 4.0)
                hi = eh_t[(blk, m // 2)][:, m % 2, :]
                lo = el_t[(blk, m // 2)][:, m % 2, :]
                nc.vector.tensor_copy(hi, ex[:])
                # alternate the residual subtract across DVE/Pool so neither
                # serial stream paces the exp-consuming matmuls
                if m % 2 == 0:
                    nc.vector.tensor_sub(lo, ex[:], hi)
                else:
                    nc.gpsimd.tensor_sub(lo, ex[:], hi)

            # scores blk0: k-outer 8-bank waves (start as xk8[0] lands)
            q0 = 0
            for half in range(2):
                ms = range(half * 8, half * 8 + 8)
                pss = {m: psp.tile([128, BQ], F32, tag=f"ps{m % 8}",
                                   name=f"psS0_{m}") for m in ms}
                for pr in range(NP):
                    for m in ms:
                        nc.tensor.matmul(
                            pss[m][:],
                            xk8_t[:, pr, :, m * 128:(m + 1) * 128],
                            qt8_t[pr][:, :, q0:q0 + BQ],
                            start=(pr == 0), stop=(pr == NP - 1),
                            perf_mode=DR)
                for m in ms:
                    exp_drain(0, m, pss[m])

            # scores blk1: 2-bank ping-pong
            q0 = BQ
            for m in range(MK):
                ps = psp.tile([128, BQ], F32, tag=f"ps{m % 2}",
                              name=f"psS1_{m}")
                for pr in range(NP):
                    nc.tensor.matmul(
                        ps[:],
                        xk8_t[:, pr, :, m * 128:(m + 1) * 128],
                        qt8_t[pr][:, :, q0:q0 + BQ],
                        start=(pr == 0), stop=(pr == NP - 1),
                        perf_mode=DR)
                exp_drain(1, m, ps)

            def zt_phase(blk):
                # Z^T/2 [e, q] = sum_s (Xv/2)^T exp: 3-pass DR, e_-outer.
                # eh-only passes run first so the el drains (one extra hop
                # behind eh) have the whole first loop to land.
                for e_ in range(EK):
                    ps = psp.tile([128, BQ], F32, tag=f"ps{2 + e_ % 2}",
                                  name=f"psZ{blk}_{e_}")
                    sl = slice(e_ * 128, (e_ + 1) * 128)
                    for tp in range(SP8):
                        nc.tensor.matmul(
                            ps[:], xvh_t[:, tp, :, sl], eh_t[(blk, tp)][:],
                            start=(tp == 0), stop=False, perf_mode=DR)
                        nc.tensor.matmul(
                            ps[:], xvl_t[:, tp, :, sl], eh_t[(blk, tp)][:],
                            start=False, stop=False, perf_mode=DR)
                    for tp in range(SP8):
                        nc.tensor.matmul(
                            ps[:], xvh_t[:, tp, :, sl], el_t[(blk, tp)][:],
                            start=False, stop=(tp == SP8 - 1), perf_mode=DR)
                    hi = zh_t[(blk, e_ // 2)][:, e_ % 2, :]
                    lo = zl_t[(blk, e_ // 2)][:, e_ % 2, :]
                    nc.scalar.copy(hi, ps[:])
                    nc.vector.tensor_sub(lo, ps[:], hi)

            def sums_phase(blk):
                # sums row = 16 * colsum(exp) via ones(16) DR matmuls,
                # then K=1 transpose to per-partition recip inputs
                # el's contribution to sums is +-2%/sqrt(2048) ~ 4e-4: skip it
                ps_row = psp.tile([128, BQ], F32, tag="ps6",
                                  name=f"psRow{blk}")
                for tp in range(SP8):
                    nc.tensor.matmul(ps_row[:], ones8[:],
                                     eh_t[(blk, tp)][:],
                                     start=(tp == 0), stop=(tp == SP8 - 1),
                                     perf_mode=DR)
                # fp16 row of 16*sums/1024 (~40, exact to 5e-4); stationary
                # for both the recip transpose and the rank-1 bv fold
                sums_sb = srp.tile([1, BQ], FP16, tag="sums_sb",
                                   name=f"sums_sb{blk}")
                nc.scalar.activation(sums_sb[:], ps_row[0:1, :],
                                     AF.Copy, scale=1.0 / 1024.0)
                sums16 = sums_sb[0:1, BQ - 128:BQ] if blk == NBLK - 1 else None
                psT = psp.tile([128, 4], F32, tag="ps7", name=f"psT{blk}")
                recips = []
                for it in range(BQ // 128):
                    nc.tensor.matmul(psT[:, it:it + 1],
                                     sums_sb[0:1, it * 128:(it + 1) * 128],
                                     ones1[:], start=True, stop=True)
                for it in range(BQ // 128):
                    rc = rcp.tile([128, 1], F32, tag=f"rc{it}",
                                  name=f"rc{blk}_{it}")
                    nc.vector.reciprocal(rc[:], psT[:, it:it + 1])
                    recips.append(rc)
                return recips, sums16

            def o_phase(blk, recips, sums16):
                # psO = 16*O via 3-pass DR over zh/zl x wvh/wvl.  One merged
                # out-DMA per i-tile except the last two (those stream
                # per-chunk so the end-of-kernel DMA chain is short).
                q0 = blk * BQ
                for it in range(BQ // 128):
                    ot = otp.tile([128, E], F32, tag=f"ot{it}",
                                  name=f"ot{blk}_{it}")
                    last_it = (blk == NBLK - 1 and it == BQ // 128 - 1)
                    tail = last_it
                    # taper the very last i-tile so the end-of-kernel
                    # Act->descgen->DMA chain covers only 128 columns
                    widths = [512, 384, 128] if last_it else [512, 512]
                    f0 = 0
                    for ci, cw in enumerate(widths):
                        final = last_it and ci == len(widths) - 1
                        ps = psp.tile([128, cw], F32, tag=f"ps{4 + ci % 2}",
                                      name=f"psO{blk}_{it}_{ci}")
                        isl = slice(it * 128, (it + 1) * 128)
                        combos = [(zh_t, wvh_t), (zh_t, wvl_t), (zl_t, wvh_t)]
                        for zp in range(NP):
                            for k, (z_src, w_src) in enumerate(combos):
                                nc.tensor.matmul(
                                    ps[:],
                                    z_src[(blk, zp)][:, :, isl],
                                    w_src[:, zp, :, f0:f0 + cw],
                                    start=(zp == 0 and k == 0),
                                    stop=(zp == NP - 1 and k == 2
                                          and not final),
                                    perf_mode=DR)
                        if final:
                            # psO += (16*sums/1024) * (1024*bv); after the
                            # recip scale this is exactly + bv
                            nc.tensor.matmul(
                                ps[:], sums16,
                                bv16_row[0:1, f0:f0 + cw],
                                start=False, stop=True)
                        nc.scalar.activation(
                            ot[:, f0:f0 + cw], ps[:],
                            AF.Copy, scale=recips[it][:])
                        if not final:
                            nc.vector.tensor_add(
                                ot[:, f0:f0 + cw],
                                ot[:, f0:f0 + cw],
                                bv_bc[:, f0:f0 + cw])
                        if tail:
                            nc.sync.dma_start(
                                out[q0 + it * 128:q0 + (it + 1) * 128,
                                    f0:f0 + cw],
                                ot[:, f0:f0 + cw])
                        f0 += cw
                    if not tail:
                        nc.sync.dma_start(
                            out[q0 + it * 128:q0 + (it + 1) * 128, :],
                            ot[:])

            zt_phase(0)
            rec0 = sums_phase(0)
            zt_phase(1)
            rec1 = sums_phase(1)
            o_phase(0, *rec0)
            o_phase(1, *rec1)

    nc.compile()
    return nc


def _get_nc():
    if "nc" not in _cached:
        _cached["nc"] = _build()
    return _cached["nc"]


def _split8(x, s):
    """fp8 hi/lo pair of s*x (hi + lo == s*x up to lo's own rounding)."""
    xs = np.asarray(x, np.float32) * s
    hi = xs.astype(ml_dtypes.float8_e4m3)
    lo = (xs - hi.astype(np.float32)).astype(ml_dtypes.float8_e4m3)
    return hi, lo


def _pack_pairs(a):
    """[R, C] fp8 -> DoubleRow pair tiles, partition-major [128, nt*2*C]."""
    r, c = a.shape
    return np.ascontiguousarray(
        a.reshape(r // 256, 2, 128, c).transpose(2, 0, 1, 3)
        .reshape(128, (r // 128) * c))


def kernel(query, key, value, Wq, bq, Wk, bk, Wv, bv, **kw):
    query = np.asarray(query, dtype=np.float32)
    key = np.asarray(key, dtype=np.float32)
    value = np.asarray(value, dtype=np.float32)
    Wq = np.asarray(Wq, dtype=np.float32)
    Wk = np.asarray(Wk, dtype=np.float32)
    Wv = np.asarray(Wv, dtype=np.float32)
    bq = np.asarray(bq, dtype=np.float32)
    bv = np.asarray(bv, dtype=np.float32)

    def _f8_one(x, s):
        return _pack_pairs((np.asarray(x, np.float32) * s)
                           .astype(ml_dtypes.float8_e4m3))

    wq8_h = _f8_one(Wq.T, 32.0)
    wk8_h = _f8_one(Wk.T, 32.0)
    wvh, wvl = _split8(Wv, 32.0)
    wvh_h, wvl_h = _pack_pairs(wvh), _pack_pairs(wvl)

    g_dev = (Wk @ bq) * 1024.0        # [E]; bk cancels in softmax
    g16_h = np.ascontiguousarray(
        (16.0 * g_dev).reshape(EK, 128).T).astype(np.float32)
    gh2_h = np.ascontiguousarray(
        (g_dev / 2048.0).reshape(EK, 128).T).astype(np.float32)
    bv_h = np.ascontiguousarray(bv.reshape(1, E))
    bv16_h = np.ascontiguousarray(
        (1024.0 * bv).reshape(1, E)).astype(np.float16)

    key8 = {}
    val8 = {}
    for b in range(B):
        key8[b] = _f8_one(key[b].T, 8.0)
        vh, vl = _split8(value[b], 0.5)
        val8[b] = (_pack_pairs(vh), _pack_pairs(vl))

    in_maps = []
    for c in range(N_CORES):
        b, h = divmod(c, 2)
        xqh, xql = _split8(query[b, h * SQ:(h + 1) * SQ, :].T, 16.0)
        in_maps.append({
            "wq8": wq8_h, "wk8": wk8_h,
            "xq8h": _pack_pairs(xqh), "xq8l": _pack_pairs(xql),
            "xk8": key8[b], "xv8h": val8[b][0], "xv8l": val8[b][1],
            "wv8h": wvh_h, "wv8l": wvl_h,
            "g16h": g16_h, "gh2": gh2_h, "bvh": bv_h, "bv16h": bv16_h,
        })

    nc = _get_nc()
    res = bass_utils.run_bass_kernel_spmd(
        nc, in_maps, core_ids=list(range(N_CORES)), **kw)

    full = np.empty((B, S, E), dtype=np.float32)
    for c in range(N_CORES):
        b, h = divmod(c, 2)
        full[b, h * SQ:(h + 1) * SQ, :] = res.results[c]["out"]
    kernel.last_results = res
    return full
